# revision 33
# baseline (speedup 1.0000x reference)
"""TRN2 Bass kernel for nn_BaseDA: 2-layer GCN on two graphs + CE loss + MMD-RBF.

v2 strategy (8 NeuronCores, SPMD), derived from the v1 trace (372us,
~230us of pre-MMD stalls):
  - Layer-1 transform is REPLICATED (each core computes z1 for all 4096
    nodes from full bf16 feature loads) -> kills the first AllGather.
  - Two AllGathers remain: h1 (node-major) for the layer-2 propagation,
    and h2 (feature-major) + stats for the MMD phase.
  - Propagation stays densified: host builds PT = (D^-1/2 (A+I) D^-1/2)^T
    column slices; 32 accumulating bf16 matmuls per graph/layer.
  - MMD: symmetry-halved supertile grid, processed as 17 QUADS of 4
    row-tiles x same column block (one [128,2048] instruction per op).
    Within a quad every tile has the same symmetry weight on every core,
    so each op's fused accum_out gives a cleanly weightable partial sum.
    Per quad: PE 4 matmuls (psi), ACT exp(psi) + exp(2 psi), DVE three
    tensor_tensor_reduce squarings (u4/u8/u16). Two quads use a DVE
    u2=u1^2 instead of the second exp to balance ACT/DVE.
  - All sign weighting, ln(softmax-denominator) and final reductions
    happen on the HOST from a [128, 96] per-core result (no ACT table
    switches on device; single exp table load at t=0).
  - rhs for the psi matmul is built raw (no on-device scaling of the
    [*, 8192] matrix): gathered features + host ones row + raw sq row.
    The bandwidth scale c is folded into the SHORT local lhs rows.
"""

import os
import numpy as np
import ml_dtypes

N = 4096
F_IN = 128
H = 64
C = 16
NEG = 0.01
NCORES = 8
NP = N // NCORES          # 512 nodes per core per graph
M2 = 2 * N                # 8192 rows/cols of the MMD kernel matrix
K_AUG = H + 2

# AG-B payload layout (bf16 words)
HW_B = 2 * H * NP                # 65536: h2 s|t feature-major
SQ_OFF = HW_B                    # 1024 bf16 sq values ([g][512])
F32_OFF = HW_B + 2 * NP          # f32 region (even bf16 offset)
NF32 = 2 + H + 6                 # s1 (s,t) + v[64] + pad to 32B multiple
AGW_B = F32_OFF + 2 * NF32

NQUAD = 17                       # 9 (half 0, x=0..8) + 8 (half 1, x=8..15)
DVE_U2 = (0, 2, 4, 6, 8, 10, 12, 14, 16)   # groups whose u2 runs on DVE
NOUT = 96                       # 34 u1/u2 accums + 4 se + 4 pk + acc col 93

BF16 = ml_dtypes.bfloat16

_CACHE = {}
LAST_EXEC_NS = None
LAST_SCOPES = None


def _install_ntff_hook():
    """The axon image lacks antenv.axon_hooks; shim it so trace=True works."""
    import sys, types
    if 'antenv.axon_hooks' in sys.modules:
        return
    mod = types.ModuleType('antenv.axon_hooks')
    mod._hook = None
    def set_axon_ntff_profile_hook(h):
        mod._hook = h
    def get_axon_ntff_profile_hook():
        return mod._hook
    mod.set_axon_ntff_profile_hook = set_axon_ntff_profile_hook
    mod.get_axon_ntff_profile_hook = get_axon_ntff_profile_hook
    sys.modules['antenv.axon_hooks'] = mod
    try:
        import antenv
        antenv.axon_hooks = mod
        from trn_agent_boot.trn_boot import _ntff_profile_via_ctypes
        set_axon_ntff_profile_hook(_ntff_profile_via_ctypes('/opt/axon/libaxon_pjrt.so'))
    except Exception:
        pass


def _build_program():
    import concourse.bass as bass
    import concourse.tile as tile
    from concourse import bacc, mybir, bass_isa

    f32 = mybir.dt.float32
    bf16 = mybir.dt.bfloat16
    Alu = mybir.AluOpType
    Act = mybir.ActivationFunctionType
    AxX = mybir.AxisListType.X

    nc = bacc.Bacc("TRN2", target_bir_lowering=False, debug=False,
                   num_devices=NCORES)

    # ---- kernel I/O ----
    ftS_d = nc.dram_tensor("ftS", [F_IN, N], bf16, kind="ExternalInput")
    ftT_d = nc.dram_tensor("ftT", [F_IN, N], bf16, kind="ExternalInput")
    ptS_d = nc.dram_tensor("ptS", [N, NP], bf16, kind="ExternalInput")
    ptT_d = nc.dram_tensor("ptT", [N, NP], bf16, kind="ExternalInput")
    w1_d = nc.dram_tensor("w1b", [F_IN, H], bf16, kind="ExternalInput")
    w2_d = nc.dram_tensor("w2b", [H, H], bf16, kind="ExternalInput")
    b1_d = nc.dram_tensor("b1", [H, 1], f32, kind="ExternalInput")
    b2_d = nc.dram_tensor("b2", [H, 1], f32, kind="ExternalInput")
    fca_d = nc.dram_tensor("fca", [H + 1, C], bf16, kind="ExternalInput")
    oh_d = nc.dram_tensor("oh", [128, 4 * C], f32, kind="ExternalInput")
    eye_d = nc.dram_tensor("eye", [H, H], bf16, kind="ExternalInput")
    cb_d = nc.dram_tensor("colbase", [1, 1], mybir.dt.int32, kind="ExternalInput")
    ones16k_d = nc.dram_tensor("ones16k", [1, 2 * M2], bf16, kind="ExternalInput")
    ones1k_d = nc.dram_tensor("ones1k", [1, 2 * NP], bf16, kind="ExternalInput")
    pm_d = nc.dram_tensor("pm_all", [128, 2 * NQUAD], bf16, kind="ExternalInput")
    out_d = nc.dram_tensor("out_vec", [128, NOUT], f32, kind="ExternalOutput")

    # ---- internal DRAM ----
    agA_in = nc.dram_tensor("agA_in", [128, 2 * 4 * H], bf16)
    agA_out = nc.dram_tensor("agA_out", [NCORES, 128, 2 * 4 * H], bf16,
                             addr_space="Shared")
    agB_in = nc.dram_tensor("agB_in", [1, AGW_B], bf16)
    agB_out = nc.dram_tensor("agB_out", [NCORES, 1, AGW_B], bf16, addr_space="Shared")
    agW_in = nc.dram_tensor("agW_in", [1, 16], bf16)
    agW_out = nc.dram_tensor("agW_out", [NCORES, 1, 16], bf16, addr_space="Shared")
    rhs_dram = nc.dram_tensor("rhs_dram", [K_AUG, 2 * M2], bf16)

    RG = [list(range(NCORES))]

    with tile.TileContext(nc) as tc:
        with tc.tile_pool(name="persist", bufs=1) as pp, \
             tc.tile_pool(name="work", bufs=2) as wp:

            # ================= constants & early setup =================
            cb_sb = pp.tile([1, 1], mybir.dt.int32, tag="cb_sb")
            nc.sync.dma_start(out=cb_sb[:], in_=cb_d.ap())
            w1_sb = pp.tile([F_IN, H], bf16, tag="w1")
            nc.sync.dma_start(out=w1_sb[:], in_=w1_d.ap())
            w2_sb = pp.tile([H, H], bf16, tag="w2")
            nc.sync.dma_start(out=w2_sb[:], in_=w2_d.ap())
            b1_sb = pp.tile([H, 1], f32, tag="b1")
            nc.sync.dma_start(out=b1_sb[:], in_=b1_d.ap())
            b2_sb = pp.tile([H, 1], f32, tag="b2")
            nc.sync.dma_start(out=b2_sb[:], in_=b2_d.ap())
            fca_sb = pp.tile([H + 1, C], bf16, tag="fca")
            nc.sync.dma_start(out=fca_sb[:], in_=fca_d.ap())
            oh_sb = pp.tile([128, 4 * C], f32, tag="oh")
            nc.sync.dma_start(out=oh_sb[:], in_=oh_d.ap())
            eye_sb = pp.tile([H, H], bf16, tag="eye")
            nc.sync.dma_start(out=eye_sb[:], in_=eye_d.ap())
            ones1k_sb = pp.tile([1, 2 * NP], bf16, tag="ones1k")
            nc.sync.dma_start(out=ones1k_sb[:], in_=ones1k_d.ap())
            pm_sb = pp.tile([128, 2 * NQUAD], bf16, tag="pm_sb")
            nc.sync.dma_start(out=pm_sb[:], in_=pm_d.ap())

            # tiny dummy AllGather at t=0: absorbs the SPMD barrier + ncfw
            # cold-start cost while the GCN phase computes
            warm_ag = pp.tile([1, 16], bf16, tag="warm_ag")
            nc.vector.memset(warm_ag[:], 0.0)
            nc.scalar.dma_start(out=agW_in.ap(), in_=warm_ag[:])
            nc.gpsimd.collective_compute(
                "AllGather", Alu.bypass, replica_groups=RG,
                ins=[agW_in.ap()], outs=[agW_out.ap()],
            )



            # rotation offset register (free-dim elements)
            with nc.gpsimd.register("colbase_reg") as cbreg:
                nc.gpsimd.reg_load(cbreg, cb_sb[0:1, 0:1])
                rot_off = nc.gpsimd.snap(cbreg)

            ones64 = pp.tile([H, 1], bf16, tag="ones64")
            nc.vector.memset(ones64[:], 1.0)
            warm_src = pp.tile([H, NP], bf16, tag="warm_src")
            nc.vector.memset(warm_src[:], 0.0)

            # result grid: [0:85) mmd accums, [85:89) se, [89:93) pk
            rgrid = pp.tile([128, NOUT], f32, tag="rgrid")
            nc.vector.memset(rgrid[:], 0.0)

            # classifier lhs (rows 0:64 filled after prop2)
            cls_lhsT = pp.tile([H + 1, NP], bf16, tag="cls_lhsT")
            nc.vector.memset(cls_lhsT[H:H + 1, :], 1.0)

            # pre-load the exp ACT table via a tiny dummy exp
            dummy = wp.tile([1, 1], f32, tag="dummy")
            nc.scalar.activation(dummy[:], warm_src[0:1, 0:1], Act.Exp)

            h2_bf = {}
            for g in "st":
                h2_bf[g] = pp.tile([H, NP], bf16, tag=f"h2_{g}", name=f"h2_{g}")

            # =================== GCN phase ===================
            with nc.named_scope("gcn"):
                with tc.tile_pool(name="gcn", bufs=1) as gp, \
                     tc.tile_pool(name="ps_z", bufs=2, space="PSUM") as psz, \
                     tc.tile_pool(name="ps_prop", bufs=2, space="PSUM") as psp, \
                     tc.tile_pool(name="ps_warm", bufs=1, space="PSUM") as psw:

                    # PE warm chain A (keeps HAM open from t~1us)
                    wps = psw.tile([H, NP], f32, tag="warm")
                    for w in range(26):
                        nc.tensor.matmul(wps[:], lhsT=warm_src[:, 0:H],
                                         rhs=warm_src[:], start=(w == 0),
                                         stop=False, skip_group_check=True)

                    def warm_fill(n):
                        for _ in range(n):
                            nc.tensor.matmul(wps[:], lhsT=warm_src[:, 0:H],
                                             rhs=warm_src[:], start=False,
                                             stop=False, skip_group_check=True)

                    # full feature loads (replicated transform)
                    ft_sb = {}
                    for g, src in (("s", ftS_d), ("t", ftT_d)):
                        t = gp.tile([F_IN, N], bf16, tag=f"ft_{g}", name=f"ft_{g}")
                        nc.sync.dma_start(out=t[:], in_=src.ap())
                        ft_sb[g] = t

                    # PT loads, 4 chunks per graph, on scalar+gpsimd queues
                    pt_sb = {}
                    for g, src, eng in (("s", ptS_d, nc.scalar), ("t", ptT_d, nc.gpsimd)):
                        t = gp.tile([128, 32 * NP], bf16, tag=f"pt_{g}", name=f"pt_{g}")
                        for c in range(4):
                            eng.dma_start(
                                out=t[:, 8 * NP * c:8 * NP * (c + 1)]
                                    .rearrange("p (k j) -> p k j", k=8),
                                in_=src.ap()[8 * 128 * c:8 * 128 * (c + 1), :]
                                    .rearrange("(k p) j -> p k j", k=8),
                            )
                        pt_sb[g] = t

                    # ---- layer 1: replicated transform z1 = X @ W1 (node-major) ----
                    z1n = {}
                    for g in "st":
                        zt = gp.tile([128, 32 * H], bf16, tag=f"z1_{g}", name=f"z1_{g}")
                        for q in range(4):   # 4 psum banks of 8 chunks
                            ps = psz.tile([128, 8 * H], f32, tag="z1ps")
                            for j in range(8):
                                ck = 8 * q + j
                                nc.tensor.matmul(
                                    ps[:, H * j:H * (j + 1)],
                                    lhsT=ft_sb[g][:, 128 * ck:128 * (ck + 1)],
                                    rhs=w1_sb[:], start=True, stop=True,
                                )
                            nc.scalar.copy(zt[:, 8 * H * q:8 * H * (q + 1)], ps[:])
                        z1n[g] = zt

                    # ---- layer 1 propagation (local columns) + bias + leaky ----
                    h1_bf = {}
                    for g in "st":
                        psH = psp.tile([H, NP], f32, tag="psH")
                        for c in range(4):
                            warm_fill(6)   # cover the PT-chunk DMA wait
                            for k in range(8 * c, 8 * c + 8):
                                nc.tensor.matmul(
                                    psH[:],
                                    lhsT=z1n[g][:, H * k:H * (k + 1)],
                                    rhs=pt_sb[g][:, NP * k:NP * (k + 1)],
                                    start=(k == 0), stop=(k == 31),
                                )
                        tsb = wp.tile([H, NP], f32, tag="hb")
                        nc.vector.tensor_scalar(tsb[:], psH[:], b1_sb[:], None, Alu.add)
                        hb = gp.tile([H, NP], bf16, tag=f"h1_{g}", name=f"h1_{g}")
                        nc.vector.scalar_tensor_tensor(hb[:], tsb[:], NEG, tsb[:],
                                                       Alu.mult, Alu.max)
                        h1_bf[g] = hb

                    # ---- transpose h1 to node-major, pack, AllGather A ----
                    h1n = gp.tile([128, 2 * 4 * H], bf16, tag="h1n")
                    for gi, g in ((0, "s"), (1, "t")):
                        for b in range(4):
                            psT = psz.tile([128, H], bf16, tag="z1ps", name=f"psT{gi}{b}")
                            nc.tensor.transpose(psT[:], h1_bf[g][:, 128 * b:128 * (b + 1)],
                                                eye_sb[:])
                            nc.scalar.copy(h1n[:, (gi * 4 + b) * H:(gi * 4 + b + 1) * H],
                                           psT[:])
                    nc.sync.dma_start(out=agA_in.ap(), in_=h1n[:])
                    nc.gpsimd.collective_compute(
                        "AllGather", Alu.bypass, replica_groups=RG,
                        ins=[agA_in.ap()], outs=[agA_out.ap()],
                    )

                    # PE warm chain B through the collective wait
                    for w in range(40):
                        nc.tensor.matmul(wps[:], lhsT=warm_src[:, 0:H],
                                         rhs=warm_src[:], start=False,
                                         stop=False, skip_group_check=True)

                    # ---- layer 2: gather z, propagate, apply W2, bias, leaky ----
                    engs = [nc.sync, nc.scalar, nc.gpsimd]
                    z_tiles = []
                    for r in range(8):
                        zr = gp.tile([128, 2 * 4 * H], bf16, tag=f"zr{r}",
                                     name=f"zr{r}")
                        engs[r % 3].dma_start(out=zr[:], in_=agA_out.ap()[r])
                        z_tiles.append(zr)
                    for gi, g in ((0, "s"), (1, "t")):
                        psA = psp.tile([H, NP], f32, tag="psH", name=f"psA_{g}")
                        for k in range(32):
                            zsrc = z_tiles[k // 4]
                            off = gi * 4 * H + (k % 4) * H
                            nc.tensor.matmul(
                                psA[:],
                                lhsT=zsrc[:, off:off + H],
                                rhs=pt_sb[g][:, NP * k:NP * (k + 1)],
                                start=(k == 0), stop=(k == 31),
                            )
                        aA = wp.tile([H, NP], bf16, tag="aA")
                        nc.vector.tensor_copy(aA[:], psA[:])
                        ps2 = psp.tile([H, NP], f32, tag="psH", name=f"ps2_{g}")
                        nc.tensor.matmul(ps2[:], lhsT=w2_sb[:], rhs=aA[:],
                                         start=True, stop=True)
                        tsb = wp.tile([H, NP], f32, tag="hb", name=f"hb2_{g}")
                        nc.vector.tensor_scalar(tsb[:], ps2[:], b2_sb[:], None, Alu.add)
                        nc.vector.scalar_tensor_tensor(h2_bf[g][:], tsb[:], NEG, tsb[:],
                                                       Alu.mult, Alu.max)

            # ============ stats + AllGather B ============
            with nc.named_scope("stats_agB"):
                with tc.tile_pool(name="ps_stat", bufs=2, space="PSUM") as psst, \
                     tc.tile_pool(name="ps_warm2", bufs=1, space="PSUM") as psw2:
                    sq_bf = pp.tile([1, 2 * NP], bf16, tag="sq_bf")
                    s1p = pp.tile([1, 2], f32, tag="s1p")
                    vpg = pp.tile([H, 2], f32, tag="vpg")
                    for gi, g in ((0, "s"), (1, "t")):
                        hsq = wp.tile([H, NP], bf16, tag="hsq")
                        nc.vector.tensor_tensor(hsq[:], h2_bf[g][:], h2_bf[g][:], Alu.mult)
                        psq = psst.tile([1, NP], f32, tag="stat")
                        nc.tensor.matmul(psq[:], lhsT=ones64[:], rhs=hsq[:],
                                         start=True, stop=True)
                        nc.scalar.activation(sq_bf[:, gi * NP:(gi + 1) * NP],
                                             psq[:], Act.Copy,
                                             accum_out=s1p[:, gi:gi + 1])
                        vscr = wp.tile([H, NP], f32, tag="vscr")
                        nc.vector.tensor_scalar(vscr[:], h2_bf[g][:], 0.0, 0.0, Alu.add,
                                                Alu.add, accum_out=vpg[:, gi:gi + 1])
                    v_part = pp.tile([H, 1], f32, tag="v_part")
                    nc.vector.tensor_reduce(v_part[:], vpg[:], AxX, Alu.add)

                    # pack payload: h2 s|t, sq, f32 stats
                    for gi, g in ((0, "s"), (1, "t")):
                        nc.sync.dma_start(
                            out=agB_in.ap()[:, gi * H * NP:(gi + 1) * H * NP]
                                .rearrange("o (f j) -> (o f) j", f=H),
                            in_=h2_bf[g][:])
                    nc.sync.dma_start(out=agB_in.ap()[:, SQ_OFF:SQ_OFF + 2 * NP],
                                      in_=sq_bf[:])
                    nc.sync.dma_start(
                        out=agB_in.ap()[:, F32_OFF:F32_OFF + 4].bitcast(f32),
                        in_=s1p[:])
                    nc.sync.dma_start(
                        out=agB_in.ap()[:, F32_OFF + 4:F32_OFF + 4 + 2 * H].bitcast(f32),
                        in_=v_part[:])
                    nc.gpsimd.collective_compute(
                        "AllGather", Alu.bypass, replica_groups=RG,
                        ins=[agB_in.ap()], outs=[agB_out.ap()],
                    )

                    # PE warm chain C + classifier during the collective
                    wps2 = psw2.tile([H, NP], f32, tag="warm2")
                    for w in range(48):
                        nc.tensor.matmul(wps2[:], lhsT=warm_src[:, 0:H],
                                         rhs=warm_src[:], start=(w == 0),
                                         stop=False, skip_group_check=True)

                    nc.vector.tensor_copy(cls_lhsT[0:H, :], h2_bf["s"][:])
                    for b in range(4):
                        psL = psst.tile([128, C], f32, tag="cls")
                        nc.tensor.matmul(psL[:], lhsT=cls_lhsT[:, 128 * b:128 * (b + 1)],
                                         rhs=fca_sb[:], start=True, stop=True)
                        esc = wp.tile([128, C], f32, tag="cls_t")
                        nc.scalar.activation(esc[:], psL[:], Act.Exp,
                                             accum_out=rgrid[:, 34 + b:35 + b])
                        pks = wp.tile([128, C], f32, tag="cls_t")
                        nc.vector.scalar_tensor_tensor(
                            pks[:], psL[:], 0.0, oh_sb[:, C * b:C * (b + 1)],
                            Alu.add, Alu.mult, accum_out=rgrid[:, 38 + b:39 + b],
                        )

            # =================== MMD phase ===================
            mp_cm = tc.tile_pool(name="mmd", bufs=1)
            mp = mp_cm.__enter__()
            with nc.named_scope("mmd_prep"):
                    st_f32 = agB_out.ap().bitcast(f32)  # [NCORES, 1, AGW_B//2]
                    FB = F32_OFF // 2

                    # ---- rhs: stage raw gathered rows in SBUF, write doubled ----
                    rhs_aug = mp.tile([K_AUG, M2], bf16, tag="rhs_aug")
                    for g in range(2):
                        nc.sync.dma_start(
                            out=rhs_aug[0:H, g * N:(g + 1) * N]
                                .rearrange("f (r j) -> f r j", r=NCORES),
                            in_=agB_out.ap()[:, :, g * H * NP:(g + 1) * H * NP]
                                .rearrange("r o (f j) -> (o f) r j", f=H),
                        )
                    nc.scalar.dma_start(
                        out=rhs_aug[H:H + 1, :], in_=ones16k_d.ap()[:, 0:M2])
                    nc.scalar.dma_start(
                        out=rhs_aug[H + 1:H + 2, :]
                            .rearrange("o (g r j) -> o g r j", g=2, r=NCORES),
                        in_=agB_out.ap()[:, :, SQ_OFF:SQ_OFF + 2 * NP]
                            .rearrange("r o (g j) -> o g r j", g=2),
                    )
                    nc.sync.dma_start(out=rhs_dram.ap()[:, 0:M2], in_=rhs_aug[:])
                    nc.scalar.dma_start(out=rhs_dram.ap()[:, M2:2 * M2], in_=rhs_aug[:])

                    # ---- global stats -> c ----
                    s1g = mp.tile([1, NCORES * 2], f32, tag="s1g")
                    nc.sync.dma_start(
                        out=s1g[:].rearrange("o (r c) -> o r c", r=NCORES),
                        in_=st_f32[:, :, FB:FB + 2].rearrange("r o c -> o r c"),
                    )
                    s1_all = mp.tile([1, 1], f32, tag="s1_all")
                    nc.vector.tensor_reduce(s1_all[:], s1g[:], AxX, Alu.add)
                    vg = mp.tile([H, NCORES], f32, tag="vg")
                    nc.sync.dma_start(
                        out=vg[:],
                        in_=st_f32[:, :, FB + 2:FB + 2 + H].rearrange("r o f -> (o f) r"),
                    )
                    v_sb = mp.tile([H, 1], f32, tag="v_sb")
                    nc.vector.tensor_reduce(v_sb[:], vg[:], AxX, Alu.add)
                    v2_sb = mp.tile([H, 1], f32, tag="v2_sb")
                    nc.vector.tensor_tensor(v2_sb[:], v_sb[:], v_sb[:], Alu.mult)
                    vv_all = mp.tile([H, 1], f32, tag="vv_all")
                    nc.gpsimd.partition_all_reduce(vv_all[:], v2_sb[:], channels=H,
                                                   reduce_op=bass_isa.ReduceOp.add)
                    # bwsum = 2*m*S1 - 2*vv ; bw_base = bwsum/(m^2-m)/4 ; c = 1/(16*bw_base)
                    sc_s1 = mp.tile([1, 1], f32, tag="sc_s1")
                    nc.vector.tensor_scalar(sc_s1[:], s1_all[:], float(2 * M2), None,
                                            Alu.mult)
                    sc_bw = mp.tile([1, 1], f32, tag="sc_bw")
                    nc.vector.scalar_tensor_tensor(sc_bw[:], vv_all[0:1, :], -2.0,
                                                   sc_s1[:], Alu.mult, Alu.add)
                    denom = float(M2) * float(M2 - 1) * 4.0
                    nc.vector.tensor_scalar(sc_bw[:], sc_bw[:], 1.0 / denom, None,
                                            Alu.mult)
                    sc_inv = mp.tile([1, 1], f32, tag="sc_inv")
                    nc.vector.reciprocal(sc_inv[:], sc_bw[:])
                    nc.vector.tensor_scalar(sc_inv[:], sc_inv[:], 1.0 / 16.0, None,
                                            Alu.mult)
                    cb = mp.tile([128, 1], f32, tag="cb")
                    nc.gpsimd.partition_broadcast(cb[:], sc_inv[:])
                    c2col = mp.tile([128, 1], f32, tag="c2col")
                    nc.vector.tensor_scalar(c2col[:], cb[:], 2.0, None, Alu.mult)
                    ncol = mp.tile([128, 1], f32, tag="ncol")
                    nc.vector.tensor_scalar(ncol[:], cb[:], -1.0, None, Alu.mult)

                    # ---- rotated rhs read (dynamic offset, 4 chunks) ----
                    rhs_rot = mp.tile([K_AUG, M2], bf16, tag="rhs_rot")
                    with tc.tile_pool(name="ps_wf", bufs=1, space="PSUM") as pswf:
                        wfp = pswf.tile([128, NP], f32, tag="wf")
                        for ch in range(4):
                            nc.gpsimd.dma_start(
                                out=rhs_rot[:, 2048 * ch:2048 * (ch + 1)],
                                in_=rhs_dram.ap()[:, bass.ds(rot_off + 2048 * ch, 2048)],
                            )
                            # warm fills gated on this chunk's arrival
                            for w in range(8):
                                nc.tensor.matmul(
                                    wfp[:],
                                    lhsT=rhs_rot[:, 2048 * ch:2048 * ch + 128],
                                    rhs=rhs_rot[:, 2048 * ch:2048 * ch + NP],
                                    start=(ch == 0 and w == 0), stop=False,
                                    skip_group_check=True,
                                )

                    # ---- lhs: c-scaled local rows (aug rows via partition-0 + DMA) ----
                    lhsT_aug = mp.tile([K_AUG, 2 * NP], bf16, tag="lhsT_aug")
                    for gi, g in ((0, "s"), (1, "t")):
                        nc.vector.tensor_scalar(lhsT_aug[0:H, gi * NP:(gi + 1) * NP],
                                                h2_bf[g][:], c2col[0:H, :], None,
                                                Alu.mult)
                    lsqn = mp.tile([1, 2 * NP], bf16, tag="lsqn")
                    nc.vector.tensor_scalar(lsqn[:], sq_bf[:], ncol[0:1, :], None,
                                            Alu.mult)
                    nc.sync.dma_start(out=lhsT_aug[H:H + 1, :], in_=lsqn[:])
                    lones = mp.tile([1, 2 * NP], bf16, tag="lones")
                    nc.vector.tensor_scalar(lones[:], ones1k_sb[:], ncol[0:1, :], None,
                                            Alu.mult)
                    nc.scalar.dma_start(out=lhsT_aug[H + 1:H + 2, :], in_=lones[:])

            with nc.named_scope("mmd_loop"):
                with tc.tile_pool(name="u_scr", bufs=3) as scr, \
                     tc.tile_pool(name="u2p", bufs=3) as u2p, \
                     tc.tile_pool(name="u4p", bufs=3) as u4p, \
                     tc.tile_pool(name="u8p", bufs=3) as u8p, \
                     tc.tile_pool(name="u16p", bufs=3) as u16p, \
                     tc.tile_pool(name="ps_q", bufs=3, space="PSUM") as psq, \
                     tc.tile_pool(name="ps_acc", bufs=1, space="PSUM") as psa:

                    # persistent pm-weighted accumulator (u2-dve/u4/u8/u16 sums)
                    acc_ps = psa.tile([1, NP], f32, tag="acc")
                    first_acc = [True]

                    def acc_reduce(utile, qi):
                        for t in range(2):
                            nc.tensor.matmul(
                                acc_ps[:], lhsT=pm_sb[:, 2 * qi + t:2 * qi + t + 1],
                                rhs=utile[:, NP * t:NP * (t + 1)],
                                start=first_acc[0], stop=False,
                                skip_group_check=True,
                            )
                            first_acc[0] = False

                    qi = 0
                    for half in range(2):
                        xs = range(0, 9) if half == 0 else range(8, 16)
                        its = (0, 2) if half == 0 else (4, 6)
                        for x in xs:
                            psG = psq.tile([128, 2 * NP], f32, tag="psG")
                            for t, it in enumerate(its):
                                nc.tensor.matmul(
                                    psG[:, NP * t:NP * (t + 1)],
                                    lhsT=lhsT_aug[:, 128 * it:128 * (it + 1)],
                                    rhs=rhs_rot[:, NP * x:NP * (x + 1)],
                                    start=True, stop=True,
                                )
                            u1 = scr.tile([128, 2 * NP], bf16, tag="u1")
                            nc.scalar.activation(
                                u1[:], psG[:], Act.Exp,
                                accum_out=rgrid[:, 2 * qi:2 * qi + 1])
                            u2 = u2p.tile([128, 2 * NP], bf16, tag="u2")
                            if qi in DVE_U2:
                                nc.vector.tensor_tensor(u2[:], u1[:], u1[:], Alu.mult)
                                acc_reduce(u2, qi)
                            else:
                                nc.scalar.activation(
                                    u2[:], psG[:], Act.Exp, scale=2.0,
                                    accum_out=rgrid[:, 2 * qi + 1:2 * qi + 2])
                            u4 = u4p.tile([128, 2 * NP], bf16, tag="u4")
                            nc.vector.tensor_tensor(u4[:], u2[:], u2[:], Alu.mult)
                            acc_reduce(u4, qi)
                            u8 = u8p.tile([128, 2 * NP], bf16, tag="u8")
                            nc.vector.tensor_tensor(u8[:], u4[:], u4[:], Alu.mult)
                            acc_reduce(u8, qi)
                            u16 = u16p.tile([128, 2 * NP], bf16, tag="u16")
                            nc.vector.tensor_tensor(u16[:], u8[:], u8[:], Alu.mult)
                            acc_reduce(u16, qi)
                            qi += 1

                    acc_sb = scr.tile([1, NP], f32, tag="acc_sb")
                    nc.scalar.activation(acc_sb[:], acc_ps[:], Act.Copy,
                                         accum_out=rgrid[0:1, 93:94])

            mp_cm.__exit__(None, None, None)
            nc.sync.dma_start(out=out_d.ap(), in_=rgrid[:])

    nc.compile()
    return nc


def _host_prep(inputs):
    """Build PT matrices + per-core input shards."""
    fs = np.ascontiguousarray(np.asarray(inputs["features_s"], np.float32))
    ft = np.ascontiguousarray(np.asarray(inputs["features_t"], np.float32))
    W1 = np.asarray(inputs["W1"], np.float32)
    W2 = np.asarray(inputs["W2"], np.float32)
    b1 = np.asarray(inputs["b1"], np.float32).reshape(H, 1)
    b2 = np.asarray(inputs["b2"], np.float32).reshape(H, 1)
    fc_w = np.asarray(inputs["fc_w"], np.float32)
    fc_b = np.asarray(inputs["fc_b"], np.float32)
    labels = np.asarray(inputs["labels_s"]).astype(np.int64)

    def build_PT(src, dst):
        src = np.asarray(src).astype(np.int64)
        dst = np.asarray(dst).astype(np.int64)
        deg = np.bincount(dst, minlength=N).astype(np.float32) + 1.0
        norm = 1.0 / np.sqrt(deg)
        AT = np.bincount(src * N + dst, minlength=N * N).astype(np.float32).reshape(N, N)
        AT[np.arange(N), np.arange(N)] += 1.0
        PT = AT * norm[None, :]
        PT *= norm[:, None]
        return PT

    PTs = build_PT(inputs["es_src"], inputs["es_dst"])
    PTt = build_PT(inputs["et_src"], inputs["et_dst"])

    fc_aug = np.concatenate([fc_w, fc_b[None, :]], axis=0).astype(BF16)
    eye = np.eye(H, dtype=np.float32).astype(BF16)

    onehot = np.zeros((N, C), np.float32)
    onehot[np.arange(N), labels] = 1.0

    ftS_T = np.ascontiguousarray(fs.T).astype(BF16)
    ftT_T = np.ascontiguousarray(ft.T).astype(BF16)
    ones16k = np.ones((1, 2 * M2), BF16)
    ones1k = np.ones((1, 2 * NP), BF16)

    in_maps = []
    for r in range(NCORES):
        sl = slice(NP * r, NP * (r + 1))
        oh_r = onehot[sl].reshape(4, 128, C).transpose(1, 0, 2).reshape(128, 4 * C)
        in_maps.append({
            "colbase": np.array([[NP * r]], np.int32),
            "ftS": ftS_T, "ftT": ftT_T,
            "ptS": np.ascontiguousarray(PTs[:, sl]).astype(BF16),
            "ptT": np.ascontiguousarray(PTt[:, sl]).astype(BF16),
            "w1b": W1.astype(BF16), "w2b": W2.astype(BF16),
            "b1": b1, "b2": b2,
            "fca": fc_aug,
            "oh": np.ascontiguousarray(oh_r),
            "eye": eye,
            "ones16k": ones16k, "ones1k": ones1k,
            "pm_all": np.ascontiguousarray(
                np.broadcast_to(np.repeat(2.0 * _quad_weights(r), 2), (128, 34))
            ).astype(BF16),
        })
    return in_maps


def _quad_weights(r):
    """Symmetry weight for each of the 17 quads on core r (host side)."""
    w = np.zeros(NQUAD, np.float64)
    qi = 0
    for half in range(2):
        xs = range(0, 9) if half == 0 else range(8, 16)
        A = r if half == 0 else r + 8
        si = 1.0 if half == 0 else -1.0
        for x in xs:
            G = (r + x) % 16
            sj = 1.0 if G < 8 else -1.0
            diag = ((G - A) % 16 == 0)
            w[qi] = si * sj * (1.0 if diag else 2.0)
            qi += 1
    return w


def kernel(**inputs):
    global LAST_EXEC_NS, LAST_SCOPES
    from concourse.bass_utils import run_bass_kernel_spmd

    trace = bool(int(os.environ.get("KBENCH_TRACE", "0")))
    if trace:
        _install_ntff_hook()

    if "nc" not in _CACHE:
        _CACHE["nc"] = _build_program()
    nc = _CACHE["nc"]

    in_maps = _host_prep(inputs)
    res = run_bass_kernel_spmd(nc, in_maps, list(range(NCORES)), trace=trace)
    LAST_EXEC_NS = res.exec_time_ns
    LAST_SCOPES = res.per_core_scope_times

    mmd_total = 0.0
    pk_total = 0.0
    lse_total = 0.0
    for r in range(NCORES):
        out = res.results[r]["out_vec"].astype(np.float64)
        w = 2.0 * _quad_weights(r)
        for q in range(NQUAD):
            mmd_total += w[q] * out[:, 2 * q:2 * q + 2].sum()
        mmd_total += out[:, 93].sum()
        se = out[:, 34:38]
        pk = out[:, 38:42]
        lse_total += np.log(se).sum()
        pk_total += pk.sum()
    class_loss = -(pk_total - lse_total) / N
    domain_loss = mmd_total / (N * N)
    return np.float32(class_loss + 0.5 * domain_loss)


# revision 34
# speedup vs baseline: 1.0528x; 1.0528x over previous
"""TRN2 Bass kernel for nn_BaseDA: 2-layer GCN on two graphs + CE loss + MMD-RBF.

v2 strategy (8 NeuronCores, SPMD), derived from the v1 trace (372us,
~230us of pre-MMD stalls):
  - Layer-1 transform is REPLICATED (each core computes z1 for all 4096
    nodes from full bf16 feature loads) -> kills the first AllGather.
  - Two AllGathers remain: h1 (node-major) for the layer-2 propagation,
    and h2 (feature-major) + stats for the MMD phase.
  - Propagation stays densified: host builds PT = (D^-1/2 (A+I) D^-1/2)^T
    column slices; 32 accumulating bf16 matmuls per graph/layer.
  - MMD: symmetry-halved supertile grid, processed as 17 QUADS of 4
    row-tiles x same column block (one [128,2048] instruction per op).
    Within a quad every tile has the same symmetry weight on every core,
    so each op's fused accum_out gives a cleanly weightable partial sum.
    Per quad: PE 4 matmuls (psi), ACT exp(psi) + exp(2 psi), DVE three
    tensor_tensor_reduce squarings (u4/u8/u16). Two quads use a DVE
    u2=u1^2 instead of the second exp to balance ACT/DVE.
  - All sign weighting, ln(softmax-denominator) and final reductions
    happen on the HOST from a [128, 96] per-core result (no ACT table
    switches on device; single exp table load at t=0).
  - rhs for the psi matmul is built raw (no on-device scaling of the
    [*, 8192] matrix): gathered features + host ones row + raw sq row.
    The bandwidth scale c is folded into the SHORT local lhs rows.
"""

import os
import numpy as np
import ml_dtypes

N = 4096
F_IN = 128
H = 64
C = 16
NEG = 0.01
NCORES = 8
NP = N // NCORES          # 512 nodes per core per graph
M2 = 2 * N                # 8192 rows/cols of the MMD kernel matrix
K_AUG = H + 2

# AG-B payload layout (bf16 words)
HW_B = 2 * H * NP                # 65536: h2 s|t feature-major
SQ_OFF = HW_B                    # 1024 bf16 sq values ([g][512])
F32_OFF = HW_B + 2 * NP          # f32 region (even bf16 offset)
NF32 = 2 + H + 6                 # s1 (s,t) + v[64] + pad to 32B multiple
AGW_B = F32_OFF + 2 * NF32

NQUAD = 17                       # 9 (half 0, x=0..8) + 8 (half 1, x=8..15)
DVE_U2 = (0, 2, 4, 6, 8, 10, 12, 14, 16)   # groups whose u2 runs on DVE
NOUT = 96                       # 34 u1/u2 accums + 4 se + 4 pk + acc col 93

BF16 = ml_dtypes.bfloat16

_CACHE = {}
LAST_EXEC_NS = None
LAST_SCOPES = None


def _install_ntff_hook():
    """The axon image lacks antenv.axon_hooks; shim it so trace=True works."""
    import sys, types
    if 'antenv.axon_hooks' in sys.modules:
        return
    mod = types.ModuleType('antenv.axon_hooks')
    mod._hook = None
    def set_axon_ntff_profile_hook(h):
        mod._hook = h
    def get_axon_ntff_profile_hook():
        return mod._hook
    mod.set_axon_ntff_profile_hook = set_axon_ntff_profile_hook
    mod.get_axon_ntff_profile_hook = get_axon_ntff_profile_hook
    sys.modules['antenv.axon_hooks'] = mod
    try:
        import antenv
        antenv.axon_hooks = mod
        from trn_agent_boot.trn_boot import _ntff_profile_via_ctypes
        set_axon_ntff_profile_hook(_ntff_profile_via_ctypes('/opt/axon/libaxon_pjrt.so'))
    except Exception:
        pass


def _build_program():
    import concourse.bass as bass
    import concourse.tile as tile
    from concourse import bacc, mybir, bass_isa

    f32 = mybir.dt.float32
    bf16 = mybir.dt.bfloat16
    Alu = mybir.AluOpType
    Act = mybir.ActivationFunctionType
    AxX = mybir.AxisListType.X

    nc = bacc.Bacc("TRN2", target_bir_lowering=False, debug=False,
                   num_devices=NCORES)

    # ---- kernel I/O ----
    ftS_d = nc.dram_tensor("ftS", [F_IN, N], bf16, kind="ExternalInput")
    ftT_d = nc.dram_tensor("ftT", [F_IN, N], bf16, kind="ExternalInput")
    ptS_d = nc.dram_tensor("ptS", [N, NP], bf16, kind="ExternalInput")
    ptT_d = nc.dram_tensor("ptT", [N, NP], bf16, kind="ExternalInput")
    w1_d = nc.dram_tensor("w1b", [F_IN, H], bf16, kind="ExternalInput")
    w2_d = nc.dram_tensor("w2b", [H, H], bf16, kind="ExternalInput")
    b1_d = nc.dram_tensor("b1", [H, 1], f32, kind="ExternalInput")
    b2_d = nc.dram_tensor("b2", [H, 1], f32, kind="ExternalInput")
    fca_d = nc.dram_tensor("fca", [H + 1, C], bf16, kind="ExternalInput")
    oh_d = nc.dram_tensor("oh", [128, 4 * C], f32, kind="ExternalInput")
    eye_d = nc.dram_tensor("eye", [H, H], bf16, kind="ExternalInput")
    cb_d = nc.dram_tensor("colbase", [1, 1], mybir.dt.int32, kind="ExternalInput")
    ones16k_d = nc.dram_tensor("ones16k", [1, 2 * M2], bf16, kind="ExternalInput")
    ones1k_d = nc.dram_tensor("ones1k", [1, 2 * NP], bf16, kind="ExternalInput")
    pm_d = nc.dram_tensor("pm_all", [128, 2 * NQUAD], bf16, kind="ExternalInput")
    out_d = nc.dram_tensor("out_vec", [128, NOUT], f32, kind="ExternalOutput")

    # ---- internal DRAM ----
    agA_in = nc.dram_tensor("agA_in", [128, 2 * 4 * H], bf16)
    agA_out = nc.dram_tensor("agA_out", [NCORES, 128, 2 * 4 * H], bf16,
                             addr_space="Shared")
    agB_in = nc.dram_tensor("agB_in", [1, AGW_B], bf16)
    agB_out = nc.dram_tensor("agB_out", [NCORES, 1, AGW_B], bf16, addr_space="Shared")
    agW_in = nc.dram_tensor("agW_in", [1, 16], bf16)
    agW_out = nc.dram_tensor("agW_out", [NCORES, 1, 16], bf16, addr_space="Shared")
    rhs_dram = nc.dram_tensor("rhs_dram", [K_AUG, 2 * M2], bf16)

    RG = [list(range(NCORES))]

    with tile.TileContext(nc) as tc:
        with tc.tile_pool(name="persist", bufs=1) as pp, \
             tc.tile_pool(name="work", bufs=2) as wp:

            # ================= constants & early setup =================
            cb_sb = pp.tile([1, 1], mybir.dt.int32, tag="cb_sb")
            nc.sync.dma_start(out=cb_sb[:], in_=cb_d.ap())
            w1_sb = pp.tile([F_IN, H], bf16, tag="w1")
            nc.sync.dma_start(out=w1_sb[:], in_=w1_d.ap())
            w2_sb = pp.tile([H, H], bf16, tag="w2")
            nc.sync.dma_start(out=w2_sb[:], in_=w2_d.ap())
            b1_sb = pp.tile([H, 1], f32, tag="b1")
            nc.sync.dma_start(out=b1_sb[:], in_=b1_d.ap())
            b2_sb = pp.tile([H, 1], f32, tag="b2")
            nc.sync.dma_start(out=b2_sb[:], in_=b2_d.ap())
            fca_sb = pp.tile([H + 1, C], bf16, tag="fca")
            nc.sync.dma_start(out=fca_sb[:], in_=fca_d.ap())
            oh_sb = pp.tile([128, 4 * C], f32, tag="oh")
            nc.sync.dma_start(out=oh_sb[:], in_=oh_d.ap())
            eye_sb = pp.tile([H, H], bf16, tag="eye")
            nc.sync.dma_start(out=eye_sb[:], in_=eye_d.ap())
            ones1k_sb = pp.tile([1, 2 * NP], bf16, tag="ones1k")
            nc.sync.dma_start(out=ones1k_sb[:], in_=ones1k_d.ap())
            pm_sb = pp.tile([128, 2 * NQUAD], bf16, tag="pm_sb")
            nc.sync.dma_start(out=pm_sb[:], in_=pm_d.ap())

            # tiny dummy AllGather at t=0: absorbs the SPMD barrier + ncfw
            # cold-start cost while the GCN phase computes
            warm_ag = pp.tile([1, 16], bf16, tag="warm_ag")
            nc.vector.memset(warm_ag[:], 0.0)
            nc.scalar.dma_start(out=agW_in.ap(), in_=warm_ag[:])
            nc.gpsimd.collective_compute(
                "AllGather", Alu.bypass, replica_groups=RG,
                ins=[agW_in.ap()], outs=[agW_out.ap()],
            )



            # rotation offset register (free-dim elements)
            with nc.gpsimd.register("colbase_reg") as cbreg:
                nc.gpsimd.reg_load(cbreg, cb_sb[0:1, 0:1])
                rot_off = nc.gpsimd.snap(cbreg)

            ones64 = pp.tile([H, 1], bf16, tag="ones64")
            nc.vector.memset(ones64[:], 1.0)
            warm_src = pp.tile([H, NP], bf16, tag="warm_src")
            nc.vector.memset(warm_src[:], 0.0)

            # result grid: [0:85) mmd accums, [85:89) se, [89:93) pk
            rgrid = pp.tile([128, NOUT], f32, tag="rgrid")
            nc.vector.memset(rgrid[:], 0.0)

            # classifier lhs (rows 0:64 filled after prop2)
            cls_lhsT = pp.tile([H + 1, NP], bf16, tag="cls_lhsT")
            nc.vector.memset(cls_lhsT[H:H + 1, :], 1.0)

            # pre-load the exp ACT table via a tiny dummy exp
            dummy = wp.tile([1, 1], f32, tag="dummy")
            nc.scalar.activation(dummy[:], warm_src[0:1, 0:1], Act.Exp)

            h2_bf = {}
            for g in "st":
                h2_bf[g] = pp.tile([H, NP], bf16, tag=f"h2_{g}", name=f"h2_{g}")

            # =================== GCN phase ===================
            with nc.named_scope("gcn"):
                with tc.tile_pool(name="gcn", bufs=1) as gp, \
                     tc.tile_pool(name="ps_z", bufs=2, space="PSUM") as psz, \
                     tc.tile_pool(name="ps_prop", bufs=2, space="PSUM") as psp, \
                     tc.tile_pool(name="ps_warm", bufs=1, space="PSUM") as psw:

                    # PE warm chain A (keeps HAM open from t~1us)
                    wps = psw.tile([H, NP], f32, tag="warm")
                    for w in range(26):
                        nc.tensor.matmul(wps[:], lhsT=warm_src[:, 0:H],
                                         rhs=warm_src[:], start=(w == 0),
                                         stop=False, skip_group_check=True)

                    def warm_fill(n):
                        for _ in range(n):
                            nc.tensor.matmul(wps[:], lhsT=warm_src[:, 0:H],
                                             rhs=warm_src[:], start=False,
                                             stop=False, skip_group_check=True)

                    # full feature loads (replicated transform)
                    ft_sb = {}
                    for g, src in (("s", ftS_d), ("t", ftT_d)):
                        t = gp.tile([F_IN, N], bf16, tag=f"ft_{g}", name=f"ft_{g}")
                        nc.sync.dma_start(out=t[:], in_=src.ap())
                        ft_sb[g] = t

                    # PT loads, 4 chunks per graph, on scalar+gpsimd queues
                    pt_sb = {}
                    for g, src, eng in (("s", ptS_d, nc.scalar), ("t", ptT_d, nc.gpsimd)):
                        t = gp.tile([128, 32 * NP], bf16, tag=f"pt_{g}", name=f"pt_{g}")
                        for c in range(4):
                            eng.dma_start(
                                out=t[:, 8 * NP * c:8 * NP * (c + 1)]
                                    .rearrange("p (k j) -> p k j", k=8),
                                in_=src.ap()[8 * 128 * c:8 * 128 * (c + 1), :]
                                    .rearrange("(k p) j -> p k j", k=8),
                            )
                        pt_sb[g] = t

                    # ---- layer 1: replicated transform z1 = X @ W1 (node-major) ----
                    z1n = {}
                    for g in "st":
                        zt = gp.tile([128, 32 * H], bf16, tag=f"z1_{g}", name=f"z1_{g}")
                        for q in range(4):   # 4 psum banks of 8 chunks
                            ps = psz.tile([128, 8 * H], f32, tag="z1ps")
                            for j in range(8):
                                ck = 8 * q + j
                                nc.tensor.matmul(
                                    ps[:, H * j:H * (j + 1)],
                                    lhsT=ft_sb[g][:, 128 * ck:128 * (ck + 1)],
                                    rhs=w1_sb[:], start=True, stop=True,
                                )
                            nc.scalar.copy(zt[:, 8 * H * q:8 * H * (q + 1)], ps[:])
                        z1n[g] = zt

                    # ---- layer 1 propagation (local columns) + bias + leaky ----
                    h1_bf = {}
                    for g in "st":
                        psH = psp.tile([H, NP], f32, tag="psH")
                        for c in range(4):
                            warm_fill(6)   # cover the PT-chunk DMA wait
                            for k in range(8 * c, 8 * c + 8):
                                nc.tensor.matmul(
                                    psH[:],
                                    lhsT=z1n[g][:, H * k:H * (k + 1)],
                                    rhs=pt_sb[g][:, NP * k:NP * (k + 1)],
                                    start=(k == 0), stop=(k == 31),
                                )
                        tsb = wp.tile([H, NP], f32, tag="hb")
                        nc.vector.tensor_scalar(tsb[:], psH[:], b1_sb[:], None, Alu.add)
                        hb = gp.tile([H, NP], bf16, tag=f"h1_{g}", name=f"h1_{g}")
                        nc.vector.scalar_tensor_tensor(hb[:], tsb[:], NEG, tsb[:],
                                                       Alu.mult, Alu.max)
                        h1_bf[g] = hb

                    # ---- transpose h1 to node-major, pack, AllGather A ----
                    h1n = gp.tile([128, 2 * 4 * H], bf16, tag="h1n")
                    for gi, g in ((0, "s"), (1, "t")):
                        for b in range(4):
                            psT = psz.tile([128, H], bf16, tag="z1ps", name=f"psT{gi}{b}")
                            nc.tensor.transpose(psT[:], h1_bf[g][:, 128 * b:128 * (b + 1)],
                                                eye_sb[:])
                            nc.scalar.copy(h1n[:, (gi * 4 + b) * H:(gi * 4 + b + 1) * H],
                                           psT[:])
                    nc.sync.dma_start(out=agA_in.ap(), in_=h1n[:])
                    nc.gpsimd.collective_compute(
                        "AllGather", Alu.bypass, replica_groups=RG,
                        ins=[agA_in.ap()], outs=[agA_out.ap()],
                    )

                    # PE warm chain B through the collective wait
                    for w in range(40):
                        nc.tensor.matmul(wps[:], lhsT=warm_src[:, 0:H],
                                         rhs=warm_src[:], start=False,
                                         stop=False, skip_group_check=True)

                    # ---- layer 2: gather z, propagate, apply W2, bias, leaky ----
                    engs = [nc.sync, nc.scalar, nc.gpsimd]
                    z_tiles = []
                    for r in range(8):
                        zr = gp.tile([128, 2 * 4 * H], bf16, tag=f"zr{r}",
                                     name=f"zr{r}")
                        engs[r % 3].dma_start(out=zr[:], in_=agA_out.ap()[r])
                        z_tiles.append(zr)
                    for gi, g in ((0, "s"), (1, "t")):
                        psA = psp.tile([H, NP], f32, tag="psH", name=f"psA_{g}")
                        for k in range(32):
                            zsrc = z_tiles[k // 4]
                            off = gi * 4 * H + (k % 4) * H
                            nc.tensor.matmul(
                                psA[:],
                                lhsT=zsrc[:, off:off + H],
                                rhs=pt_sb[g][:, NP * k:NP * (k + 1)],
                                start=(k == 0), stop=(k == 31),
                            )
                        aA = wp.tile([H, NP], bf16, tag="aA")
                        nc.vector.tensor_copy(aA[:], psA[:])
                        ps2 = psp.tile([H, NP], f32, tag="psH", name=f"ps2_{g}")
                        nc.tensor.matmul(ps2[:], lhsT=w2_sb[:], rhs=aA[:],
                                         start=True, stop=True)
                        tsb = wp.tile([H, NP], f32, tag="hb", name=f"hb2_{g}")
                        nc.vector.tensor_scalar(tsb[:], ps2[:], b2_sb[:], None, Alu.add)
                        nc.vector.scalar_tensor_tensor(h2_bf[g][:], tsb[:], NEG, tsb[:],
                                                       Alu.mult, Alu.max)

            # ============ stats + AllGather B ============
            with nc.named_scope("stats_agB"):
                with tc.tile_pool(name="ps_stat", bufs=2, space="PSUM") as psst, \
                     tc.tile_pool(name="ps_warm2", bufs=1, space="PSUM") as psw2:
                    sq_bf = pp.tile([1, 2 * NP], bf16, tag="sq_bf")
                    s1p = pp.tile([1, 2], f32, tag="s1p")
                    vpg = pp.tile([H, 2], f32, tag="vpg")
                    for gi, g in ((0, "s"), (1, "t")):
                        hsq = wp.tile([H, NP], bf16, tag="hsq")
                        nc.vector.tensor_tensor(hsq[:], h2_bf[g][:], h2_bf[g][:], Alu.mult)
                        psq = psst.tile([1, NP], f32, tag="stat")
                        nc.tensor.matmul(psq[:], lhsT=ones64[:], rhs=hsq[:],
                                         start=True, stop=True)
                        nc.scalar.activation(sq_bf[:, gi * NP:(gi + 1) * NP],
                                             psq[:], Act.Copy,
                                             accum_out=s1p[:, gi:gi + 1])
                        vscr = wp.tile([H, NP], f32, tag="vscr")
                        nc.vector.tensor_scalar(vscr[:], h2_bf[g][:], 0.0, 0.0, Alu.add,
                                                Alu.add, accum_out=vpg[:, gi:gi + 1])
                    v_part = pp.tile([H, 1], f32, tag="v_part")
                    nc.vector.tensor_reduce(v_part[:], vpg[:], AxX, Alu.add)

                    # pack payload: h2 s|t, sq, f32 stats
                    for gi, g in ((0, "s"), (1, "t")):
                        nc.sync.dma_start(
                            out=agB_in.ap()[:, gi * H * NP:(gi + 1) * H * NP]
                                .rearrange("o (f j) -> (o f) j", f=H),
                            in_=h2_bf[g][:])
                    nc.sync.dma_start(out=agB_in.ap()[:, SQ_OFF:SQ_OFF + 2 * NP],
                                      in_=sq_bf[:])
                    nc.sync.dma_start(
                        out=agB_in.ap()[:, F32_OFF:F32_OFF + 4].bitcast(f32),
                        in_=s1p[:])
                    nc.sync.dma_start(
                        out=agB_in.ap()[:, F32_OFF + 4:F32_OFF + 4 + 2 * H].bitcast(f32),
                        in_=v_part[:])
                    nc.gpsimd.collective_compute(
                        "AllGather", Alu.bypass, replica_groups=RG,
                        ins=[agB_in.ap()], outs=[agB_out.ap()],
                    )

                    # PE warm chain C + classifier during the collective
                    wps2 = psw2.tile([H, NP], f32, tag="warm2")
                    for w in range(48):
                        nc.tensor.matmul(wps2[:], lhsT=warm_src[:, 0:H],
                                         rhs=warm_src[:], start=(w == 0),
                                         stop=False, skip_group_check=True)

                    nc.vector.tensor_copy(cls_lhsT[0:H, :], h2_bf["s"][:])
                    for b in range(4):
                        psL = psst.tile([128, C], f32, tag="cls")
                        nc.tensor.matmul(psL[:], lhsT=cls_lhsT[:, 128 * b:128 * (b + 1)],
                                         rhs=fca_sb[:], start=True, stop=True)
                        esc = wp.tile([128, C], f32, tag="cls_t")
                        nc.scalar.activation(esc[:], psL[:], Act.Exp,
                                             accum_out=rgrid[:, 34 + b:35 + b])
                        pks = wp.tile([128, C], f32, tag="cls_t")
                        nc.vector.scalar_tensor_tensor(
                            pks[:], psL[:], 0.0, oh_sb[:, C * b:C * (b + 1)],
                            Alu.add, Alu.mult, accum_out=rgrid[:, 38 + b:39 + b],
                        )

            # =================== MMD phase ===================
            mp_cm = tc.tile_pool(name="mmd", bufs=1)
            mp = mp_cm.__enter__()
            with nc.named_scope("mmd_prep"):
                    st_f32 = agB_out.ap().bitcast(f32)  # [NCORES, 1, AGW_B//2]
                    FB = F32_OFF // 2

                    # ---- rhs: stage raw gathered rows in SBUF, write doubled ----
                    rhs_aug = mp.tile([K_AUG, M2], bf16, tag="rhs_aug")
                    for g in range(2):
                        nc.sync.dma_start(
                            out=rhs_aug[0:H, g * N:(g + 1) * N]
                                .rearrange("f (r j) -> f r j", r=NCORES),
                            in_=agB_out.ap()[:, :, g * H * NP:(g + 1) * H * NP]
                                .rearrange("r o (f j) -> (o f) r j", f=H),
                        )
                    nc.scalar.dma_start(
                        out=rhs_aug[H:H + 1, :], in_=ones16k_d.ap()[:, 0:M2])
                    nc.scalar.dma_start(
                        out=rhs_aug[H + 1:H + 2, :]
                            .rearrange("o (g r j) -> o g r j", g=2, r=NCORES),
                        in_=agB_out.ap()[:, :, SQ_OFF:SQ_OFF + 2 * NP]
                            .rearrange("r o (g j) -> o g r j", g=2),
                    )
                    nc.sync.dma_start(out=rhs_dram.ap()[:, 0:M2], in_=rhs_aug[:])
                    nc.scalar.dma_start(out=rhs_dram.ap()[:, M2:2 * M2], in_=rhs_aug[:])

                    # ---- global stats -> c ----
                    s1g = mp.tile([1, NCORES * 2], f32, tag="s1g")
                    nc.sync.dma_start(
                        out=s1g[:].rearrange("o (r c) -> o r c", r=NCORES),
                        in_=st_f32[:, :, FB:FB + 2].rearrange("r o c -> o r c"),
                    )
                    s1_all = mp.tile([1, 1], f32, tag="s1_all")
                    nc.vector.tensor_reduce(s1_all[:], s1g[:], AxX, Alu.add)
                    vg = mp.tile([H, NCORES], f32, tag="vg")
                    nc.sync.dma_start(
                        out=vg[:],
                        in_=st_f32[:, :, FB + 2:FB + 2 + H].rearrange("r o f -> (o f) r"),
                    )
                    v_sb = mp.tile([H, 1], f32, tag="v_sb")
                    nc.vector.tensor_reduce(v_sb[:], vg[:], AxX, Alu.add)
                    v2_sb = mp.tile([H, 1], f32, tag="v2_sb")
                    nc.vector.tensor_tensor(v2_sb[:], v_sb[:], v_sb[:], Alu.mult)
                    vv_all = mp.tile([H, 1], f32, tag="vv_all")
                    nc.gpsimd.partition_all_reduce(vv_all[:], v2_sb[:], channels=H,
                                                   reduce_op=bass_isa.ReduceOp.add)
                    # bwsum = 2*m*S1 - 2*vv ; bw_base = bwsum/(m^2-m)/4 ; c = 1/(16*bw_base)
                    sc_s1 = mp.tile([1, 1], f32, tag="sc_s1")
                    nc.vector.tensor_scalar(sc_s1[:], s1_all[:], float(2 * M2), None,
                                            Alu.mult)
                    sc_bw = mp.tile([1, 1], f32, tag="sc_bw")
                    nc.vector.scalar_tensor_tensor(sc_bw[:], vv_all[0:1, :], -2.0,
                                                   sc_s1[:], Alu.mult, Alu.add)
                    denom = float(M2) * float(M2 - 1) * 4.0
                    nc.vector.tensor_scalar(sc_bw[:], sc_bw[:], 1.0 / denom, None,
                                            Alu.mult)
                    sc_inv = mp.tile([1, 1], f32, tag="sc_inv")
                    nc.vector.reciprocal(sc_inv[:], sc_bw[:])
                    nc.vector.tensor_scalar(sc_inv[:], sc_inv[:], 1.0 / 16.0, None,
                                            Alu.mult)
                    cb = mp.tile([128, 1], f32, tag="cb")
                    nc.gpsimd.partition_broadcast(cb[:], sc_inv[:])
                    c2col = mp.tile([128, 1], f32, tag="c2col")
                    nc.vector.tensor_scalar(c2col[:], cb[:], 2.0, None, Alu.mult)
                    ncol = mp.tile([128, 1], f32, tag="ncol")
                    nc.vector.tensor_scalar(ncol[:], cb[:], -1.0, None, Alu.mult)

                    # ---- rotated rhs read (dynamic offset, 4 chunks) ----
                    rhs_rot = mp.tile([K_AUG, M2], bf16, tag="rhs_rot")
                    for ch in range(4):
                        nc.gpsimd.dma_start(
                            out=rhs_rot[:, 2048 * ch:2048 * (ch + 1)],
                            in_=rhs_dram.ap()[:, bass.ds(rot_off + 2048 * ch, 2048)],
                        )

                    # ---- lhs: c-scaled local rows (aug rows via partition-0 + DMA) ----
                    lhsT_aug = mp.tile([K_AUG, 2 * NP], bf16, tag="lhsT_aug")
                    for gi, g in ((0, "s"), (1, "t")):
                        nc.vector.tensor_scalar(lhsT_aug[0:H, gi * NP:(gi + 1) * NP],
                                                h2_bf[g][:], c2col[0:H, :], None,
                                                Alu.mult)
                    lsqn = mp.tile([1, 2 * NP], bf16, tag="lsqn")
                    nc.vector.tensor_scalar(lsqn[:], sq_bf[:], ncol[0:1, :], None,
                                            Alu.mult)
                    nc.sync.dma_start(out=lhsT_aug[H:H + 1, :], in_=lsqn[:])
                    lones = mp.tile([1, 2 * NP], bf16, tag="lones")
                    nc.vector.tensor_scalar(lones[:], ones1k_sb[:], ncol[0:1, :], None,
                                            Alu.mult)
                    nc.scalar.dma_start(out=lhsT_aug[H + 1:H + 2, :], in_=lones[:])

            with nc.named_scope("mmd_loop"):
                with tc.tile_pool(name="u_scr", bufs=3) as scr, \
                     tc.tile_pool(name="u2p", bufs=3) as u2p, \
                     tc.tile_pool(name="u4p", bufs=3) as u4p, \
                     tc.tile_pool(name="u8p", bufs=3) as u8p, \
                     tc.tile_pool(name="u16p", bufs=3) as u16p, \
                     tc.tile_pool(name="ps_q", bufs=3, space="PSUM") as psq, \
                     tc.tile_pool(name="ps_acc", bufs=1, space="PSUM") as psa:

                    # persistent pm-weighted accumulator (u2-dve/u4/u8/u16 sums)
                    acc_ps = psa.tile([1, NP], f32, tag="acc")
                    first_acc = [True]

                    def acc_reduce(utile, qi):
                        for t in range(2):
                            nc.tensor.matmul(
                                acc_ps[:], lhsT=pm_sb[:, 2 * qi + t:2 * qi + t + 1],
                                rhs=utile[:, NP * t:NP * (t + 1)],
                                start=first_acc[0], stop=False,
                                skip_group_check=True,
                            )
                            first_acc[0] = False

                    qi = 0
                    for half in range(2):
                        xs = range(0, 9) if half == 0 else range(8, 16)
                        its = (0, 2) if half == 0 else (4, 6)
                        for x in xs:
                            psG = psq.tile([128, 2 * NP], f32, tag="psG")
                            for t, it in enumerate(its):
                                nc.tensor.matmul(
                                    psG[:, NP * t:NP * (t + 1)],
                                    lhsT=lhsT_aug[:, 128 * it:128 * (it + 1)],
                                    rhs=rhs_rot[:, NP * x:NP * (x + 1)],
                                    start=True, stop=True,
                                )
                            u1 = scr.tile([128, 2 * NP], bf16, tag="u1")
                            nc.scalar.activation(
                                u1[:], psG[:], Act.Exp,
                                accum_out=rgrid[:, 2 * qi:2 * qi + 1])
                            u2 = u2p.tile([128, 2 * NP], bf16, tag="u2")
                            if qi in DVE_U2:
                                nc.vector.tensor_tensor(u2[:], u1[:], u1[:], Alu.mult)
                                acc_reduce(u2, qi)
                            else:
                                nc.scalar.activation(
                                    u2[:], psG[:], Act.Exp, scale=2.0,
                                    accum_out=rgrid[:, 2 * qi + 1:2 * qi + 2])
                            u4 = u4p.tile([128, 2 * NP], bf16, tag="u4")
                            nc.vector.tensor_tensor(u4[:], u2[:], u2[:], Alu.mult)
                            acc_reduce(u4, qi)
                            u8 = u8p.tile([128, 2 * NP], bf16, tag="u8")
                            nc.vector.tensor_tensor(u8[:], u4[:], u4[:], Alu.mult)
                            acc_reduce(u8, qi)
                            u16 = u16p.tile([128, 2 * NP], bf16, tag="u16")
                            nc.vector.tensor_tensor(u16[:], u8[:], u8[:], Alu.mult)
                            acc_reduce(u16, qi)
                            qi += 1

                    acc_sb = scr.tile([1, NP], f32, tag="acc_sb")
                    nc.scalar.activation(acc_sb[:], acc_ps[:], Act.Copy,
                                         accum_out=rgrid[0:1, 93:94])

            mp_cm.__exit__(None, None, None)
            nc.sync.dma_start(out=out_d.ap(), in_=rgrid[:])

    nc.compile()
    return nc


def _host_prep(inputs):
    """Build PT matrices + per-core input shards."""
    fs = np.ascontiguousarray(np.asarray(inputs["features_s"], np.float32))
    ft = np.ascontiguousarray(np.asarray(inputs["features_t"], np.float32))
    W1 = np.asarray(inputs["W1"], np.float32)
    W2 = np.asarray(inputs["W2"], np.float32)
    b1 = np.asarray(inputs["b1"], np.float32).reshape(H, 1)
    b2 = np.asarray(inputs["b2"], np.float32).reshape(H, 1)
    fc_w = np.asarray(inputs["fc_w"], np.float32)
    fc_b = np.asarray(inputs["fc_b"], np.float32)
    labels = np.asarray(inputs["labels_s"]).astype(np.int64)

    def build_PT(src, dst):
        src = np.asarray(src).astype(np.int64)
        dst = np.asarray(dst).astype(np.int64)
        deg = np.bincount(dst, minlength=N).astype(np.float32) + 1.0
        norm = 1.0 / np.sqrt(deg)
        AT = np.bincount(src * N + dst, minlength=N * N).astype(np.float32).reshape(N, N)
        AT[np.arange(N), np.arange(N)] += 1.0
        PT = AT * norm[None, :]
        PT *= norm[:, None]
        return PT

    PTs = build_PT(inputs["es_src"], inputs["es_dst"])
    PTt = build_PT(inputs["et_src"], inputs["et_dst"])

    fc_aug = np.concatenate([fc_w, fc_b[None, :]], axis=0).astype(BF16)
    eye = np.eye(H, dtype=np.float32).astype(BF16)

    onehot = np.zeros((N, C), np.float32)
    onehot[np.arange(N), labels] = 1.0

    ftS_T = np.ascontiguousarray(fs.T).astype(BF16)
    ftT_T = np.ascontiguousarray(ft.T).astype(BF16)
    ones16k = np.ones((1, 2 * M2), BF16)
    ones1k = np.ones((1, 2 * NP), BF16)

    in_maps = []
    for r in range(NCORES):
        sl = slice(NP * r, NP * (r + 1))
        oh_r = onehot[sl].reshape(4, 128, C).transpose(1, 0, 2).reshape(128, 4 * C)
        in_maps.append({
            "colbase": np.array([[NP * r]], np.int32),
            "ftS": ftS_T, "ftT": ftT_T,
            "ptS": np.ascontiguousarray(PTs[:, sl]).astype(BF16),
            "ptT": np.ascontiguousarray(PTt[:, sl]).astype(BF16),
            "w1b": W1.astype(BF16), "w2b": W2.astype(BF16),
            "b1": b1, "b2": b2,
            "fca": fc_aug,
            "oh": np.ascontiguousarray(oh_r),
            "eye": eye,
            "ones16k": ones16k, "ones1k": ones1k,
            "pm_all": np.ascontiguousarray(
                np.broadcast_to(np.repeat(2.0 * _quad_weights(r), 2), (128, 34))
            ).astype(BF16),
        })
    return in_maps


def _quad_weights(r):
    """Symmetry weight for each of the 17 quads on core r (host side)."""
    w = np.zeros(NQUAD, np.float64)
    qi = 0
    for half in range(2):
        xs = range(0, 9) if half == 0 else range(8, 16)
        A = r if half == 0 else r + 8
        si = 1.0 if half == 0 else -1.0
        for x in xs:
            G = (r + x) % 16
            sj = 1.0 if G < 8 else -1.0
            diag = ((G - A) % 16 == 0)
            w[qi] = si * sj * (1.0 if diag else 2.0)
            qi += 1
    return w


def kernel(**inputs):
    global LAST_EXEC_NS, LAST_SCOPES
    from concourse.bass_utils import run_bass_kernel_spmd

    trace = bool(int(os.environ.get("KBENCH_TRACE", "0")))
    if trace:
        _install_ntff_hook()

    if "nc" not in _CACHE:
        _CACHE["nc"] = _build_program()
    nc = _CACHE["nc"]

    in_maps = _host_prep(inputs)
    res = run_bass_kernel_spmd(nc, in_maps, list(range(NCORES)), trace=trace)
    LAST_EXEC_NS = res.exec_time_ns
    LAST_SCOPES = res.per_core_scope_times

    mmd_total = 0.0
    pk_total = 0.0
    lse_total = 0.0
    for r in range(NCORES):
        out = res.results[r]["out_vec"].astype(np.float64)
        w = 2.0 * _quad_weights(r)
        for q in range(NQUAD):
            mmd_total += w[q] * out[:, 2 * q:2 * q + 2].sum()
        mmd_total += out[:, 93].sum()
        se = out[:, 34:38]
        pk = out[:, 38:42]
        lse_total += np.log(se).sum()
        pk_total += pk.sum()
    class_loss = -(pk_total - lse_total) / N
    domain_loss = mmd_total / (N * N)
    return np.float32(class_loss + 0.5 * domain_loss)


# revision 35
# speedup vs baseline: 1.0859x; 1.0315x over previous
"""TRN2 Bass kernel for nn_BaseDA: 2-layer GCN on two graphs + CE loss + MMD-RBF.

v2 strategy (8 NeuronCores, SPMD), derived from the v1 trace (372us,
~230us of pre-MMD stalls):
  - Layer-1 transform is REPLICATED (each core computes z1 for all 4096
    nodes from full bf16 feature loads) -> kills the first AllGather.
  - Two AllGathers remain: h1 (node-major) for the layer-2 propagation,
    and h2 (feature-major) + stats for the MMD phase.
  - Propagation stays densified: host builds PT = (D^-1/2 (A+I) D^-1/2)^T
    column slices; 32 accumulating bf16 matmuls per graph/layer.
  - MMD: symmetry-halved supertile grid, processed as 17 QUADS of 4
    row-tiles x same column block (one [128,2048] instruction per op).
    Within a quad every tile has the same symmetry weight on every core,
    so each op's fused accum_out gives a cleanly weightable partial sum.
    Per quad: PE 4 matmuls (psi), ACT exp(psi) + exp(2 psi), DVE three
    tensor_tensor_reduce squarings (u4/u8/u16). Two quads use a DVE
    u2=u1^2 instead of the second exp to balance ACT/DVE.
  - All sign weighting, ln(softmax-denominator) and final reductions
    happen on the HOST from a [128, 96] per-core result (no ACT table
    switches on device; single exp table load at t=0).
  - rhs for the psi matmul is built raw (no on-device scaling of the
    [*, 8192] matrix): gathered features + host ones row + raw sq row.
    The bandwidth scale c is folded into the SHORT local lhs rows.
"""

import os
import numpy as np
import ml_dtypes

N = 4096
F_IN = 128
H = 64
C = 16
NEG = 0.01
NCORES = 8
NP = N // NCORES          # 512 nodes per core per graph
M2 = 2 * N                # 8192 rows/cols of the MMD kernel matrix
K_AUG = H + 2

# AG-B payload layout (bf16 words)
HW_B = 2 * H * NP                # 65536: h2 s|t feature-major
SQ_OFF = HW_B                    # 1024 bf16 sq values ([g][512])
F32_OFF = HW_B + 2 * NP          # f32 region (even bf16 offset)
NF32 = 2 + H + 6                 # s1 (s,t) + v[64] + pad to 32B multiple
AGW_B = F32_OFF + 2 * NF32

NQUAD = 17                       # 9 (half 0, x=0..8) + 8 (half 1, x=8..15)
DVE_U2 = (0, 2, 4, 6, 8, 10, 12, 14, 16)   # groups whose u2 runs on DVE
NOUT = 96                       # 34 u1/u2 accums + 4 se + 4 pk + acc col 93

BF16 = ml_dtypes.bfloat16

_CACHE = {}
LAST_EXEC_NS = None
LAST_SCOPES = None


def _install_ntff_hook():
    """The axon image lacks antenv.axon_hooks; shim it so trace=True works."""
    import sys, types
    if 'antenv.axon_hooks' in sys.modules:
        return
    mod = types.ModuleType('antenv.axon_hooks')
    mod._hook = None
    def set_axon_ntff_profile_hook(h):
        mod._hook = h
    def get_axon_ntff_profile_hook():
        return mod._hook
    mod.set_axon_ntff_profile_hook = set_axon_ntff_profile_hook
    mod.get_axon_ntff_profile_hook = get_axon_ntff_profile_hook
    sys.modules['antenv.axon_hooks'] = mod
    try:
        import antenv
        antenv.axon_hooks = mod
        from trn_agent_boot.trn_boot import _ntff_profile_via_ctypes
        set_axon_ntff_profile_hook(_ntff_profile_via_ctypes('/opt/axon/libaxon_pjrt.so'))
    except Exception:
        pass


def _build_program():
    import concourse.bass as bass
    import concourse.tile as tile
    from concourse import bacc, mybir, bass_isa

    f32 = mybir.dt.float32
    bf16 = mybir.dt.bfloat16
    Alu = mybir.AluOpType
    Act = mybir.ActivationFunctionType
    AxX = mybir.AxisListType.X

    nc = bacc.Bacc("TRN2", target_bir_lowering=False, debug=False,
                   num_devices=NCORES)

    # ---- kernel I/O ----
    ftS_d = nc.dram_tensor("ftS", [F_IN, N], bf16, kind="ExternalInput")
    ftT_d = nc.dram_tensor("ftT", [F_IN, N], bf16, kind="ExternalInput")
    ptS_d = nc.dram_tensor("ptS", [N, NP], bf16, kind="ExternalInput")
    ptT_d = nc.dram_tensor("ptT", [N, NP], bf16, kind="ExternalInput")
    w1_d = nc.dram_tensor("w1b", [F_IN, H], bf16, kind="ExternalInput")
    w2_d = nc.dram_tensor("w2b", [H, H], bf16, kind="ExternalInput")
    b1_d = nc.dram_tensor("b1", [H, 1], f32, kind="ExternalInput")
    b2_d = nc.dram_tensor("b2", [H, 1], f32, kind="ExternalInput")
    fca_d = nc.dram_tensor("fca", [H + 1, C], bf16, kind="ExternalInput")
    oh_d = nc.dram_tensor("oh", [128, 4 * C], f32, kind="ExternalInput")
    eye_d = nc.dram_tensor("eye", [H, H], bf16, kind="ExternalInput")
    cb_d = nc.dram_tensor("colbase", [1, 1], mybir.dt.int32, kind="ExternalInput")
    ones16k_d = nc.dram_tensor("ones16k", [1, 2 * M2], bf16, kind="ExternalInput")
    ones1k_d = nc.dram_tensor("ones1k", [1, 2 * NP], bf16, kind="ExternalInput")
    pm_d = nc.dram_tensor("pm_all", [128, 2 * NQUAD], bf16, kind="ExternalInput")
    out_d = nc.dram_tensor("out_vec", [128, NOUT], f32, kind="ExternalOutput")

    # ---- internal DRAM ----
    agA_in = nc.dram_tensor("agA_in", [128, 2 * 4 * H], bf16)
    agA_out = nc.dram_tensor("agA_out", [NCORES, 128, 2 * 4 * H], bf16,
                             addr_space="Shared")
    agB_in = nc.dram_tensor("agB_in", [1, AGW_B], bf16)
    agB_out = nc.dram_tensor("agB_out", [NCORES, 1, AGW_B], bf16, addr_space="Shared")
    agW_in = nc.dram_tensor("agW_in", [1, 16], bf16)
    agW_out = nc.dram_tensor("agW_out", [NCORES, 1, 16], bf16, addr_space="Shared")
    rhs_dram = nc.dram_tensor("rhs_dram", [K_AUG, 2 * M2], bf16)

    RG = [list(range(NCORES))]

    with tile.TileContext(nc) as tc:
        with tc.tile_pool(name="persist", bufs=1) as pp, \
             tc.tile_pool(name="work", bufs=2) as wp:

            # ================= constants & early setup =================
            cb_sb = pp.tile([1, 1], mybir.dt.int32, tag="cb_sb")
            nc.sync.dma_start(out=cb_sb[:], in_=cb_d.ap())
            w1_sb = pp.tile([F_IN, H], bf16, tag="w1")
            nc.sync.dma_start(out=w1_sb[:], in_=w1_d.ap())
            w2_sb = pp.tile([H, H], bf16, tag="w2")
            nc.sync.dma_start(out=w2_sb[:], in_=w2_d.ap())
            b1_sb = pp.tile([H, 1], f32, tag="b1")
            nc.sync.dma_start(out=b1_sb[:], in_=b1_d.ap())
            b2_sb = pp.tile([H, 1], f32, tag="b2")
            nc.sync.dma_start(out=b2_sb[:], in_=b2_d.ap())
            fca_sb = pp.tile([H + 1, C], bf16, tag="fca")
            nc.sync.dma_start(out=fca_sb[:], in_=fca_d.ap())
            oh_sb = pp.tile([128, 4 * C], f32, tag="oh")
            nc.sync.dma_start(out=oh_sb[:], in_=oh_d.ap())
            eye_sb = pp.tile([H, H], bf16, tag="eye")
            nc.sync.dma_start(out=eye_sb[:], in_=eye_d.ap())
            ones1k_sb = pp.tile([1, 2 * NP], bf16, tag="ones1k")
            nc.sync.dma_start(out=ones1k_sb[:], in_=ones1k_d.ap())
            pm_sb = pp.tile([128, 2 * NQUAD], bf16, tag="pm_sb")
            nc.sync.dma_start(out=pm_sb[:], in_=pm_d.ap())

            # tiny dummy AllGather at t=0: absorbs the SPMD barrier + ncfw
            # cold-start cost while the GCN phase computes
            warm_ag = pp.tile([1, 16], bf16, tag="warm_ag")
            nc.vector.memset(warm_ag[:], 0.0)
            nc.scalar.dma_start(out=agW_in.ap(), in_=warm_ag[:])
            nc.gpsimd.collective_compute(
                "AllGather", Alu.bypass, replica_groups=RG,
                ins=[agW_in.ap()], outs=[agW_out.ap()],
            )



            # rotation offset register (free-dim elements)
            with nc.sync.register("colbase_reg") as cbreg:
                nc.sync.reg_load(cbreg, cb_sb[0:1, 0:1])
                rot_off = nc.sync.snap(cbreg)
            with nc.scalar.register("colbase_reg2") as cbreg2:
                nc.scalar.reg_load(cbreg2, cb_sb[0:1, 0:1])
                rot_off2 = nc.scalar.snap(cbreg2)

            ones64 = pp.tile([H, 1], bf16, tag="ones64")
            nc.vector.memset(ones64[:], 1.0)
            warm_src = pp.tile([H, NP], bf16, tag="warm_src")
            nc.vector.memset(warm_src[:], 0.0)

            # result grid: [0:85) mmd accums, [85:89) se, [89:93) pk
            rgrid = pp.tile([128, NOUT], f32, tag="rgrid")
            nc.vector.memset(rgrid[:], 0.0)

            # classifier lhs (rows 0:64 filled after prop2)
            cls_lhsT = pp.tile([H + 1, NP], bf16, tag="cls_lhsT")
            nc.vector.memset(cls_lhsT[H:H + 1, :], 1.0)

            # pre-load the exp ACT table via a tiny dummy exp
            dummy = wp.tile([1, 1], f32, tag="dummy")
            nc.scalar.activation(dummy[:], warm_src[0:1, 0:1], Act.Exp)

            h2_bf = {}
            for g in "st":
                h2_bf[g] = pp.tile([H, NP], bf16, tag=f"h2_{g}", name=f"h2_{g}")

            # =================== GCN phase ===================
            with nc.named_scope("gcn"):
                with tc.tile_pool(name="gcn", bufs=1) as gp, \
                     tc.tile_pool(name="ps_z", bufs=2, space="PSUM") as psz, \
                     tc.tile_pool(name="ps_prop", bufs=2, space="PSUM") as psp, \
                     tc.tile_pool(name="ps_warm", bufs=1, space="PSUM") as psw:

                    # PE warm chain A (keeps HAM open from t~1us)
                    wps = psw.tile([H, NP], f32, tag="warm")
                    for w in range(26):
                        nc.tensor.matmul(wps[:], lhsT=warm_src[:, 0:H],
                                         rhs=warm_src[:], start=(w == 0),
                                         stop=False, skip_group_check=True)

                    def warm_fill(n):
                        for _ in range(n):
                            nc.tensor.matmul(wps[:], lhsT=warm_src[:, 0:H],
                                             rhs=warm_src[:], start=False,
                                             stop=False, skip_group_check=True)

                    # full feature loads (replicated transform)
                    ft_sb = {}
                    for g, src in (("s", ftS_d), ("t", ftT_d)):
                        t = gp.tile([F_IN, N], bf16, tag=f"ft_{g}", name=f"ft_{g}")
                        nc.sync.dma_start(out=t[:], in_=src.ap())
                        ft_sb[g] = t

                    # PT loads, 4 chunks per graph, on scalar+gpsimd queues
                    pt_sb = {}
                    for g, src, eng in (("s", ptS_d, nc.scalar), ("t", ptT_d, nc.gpsimd)):
                        t = gp.tile([128, 32 * NP], bf16, tag=f"pt_{g}", name=f"pt_{g}")
                        for c in range(4):
                            eng.dma_start(
                                out=t[:, 8 * NP * c:8 * NP * (c + 1)]
                                    .rearrange("p (k j) -> p k j", k=8),
                                in_=src.ap()[8 * 128 * c:8 * 128 * (c + 1), :]
                                    .rearrange("(k p) j -> p k j", k=8),
                            )
                        pt_sb[g] = t

                    # ---- layer 1: replicated transform z1 = X @ W1 (node-major) ----
                    z1n = {}
                    for g in "st":
                        zt = gp.tile([128, 32 * H], bf16, tag=f"z1_{g}", name=f"z1_{g}")
                        for q in range(4):   # 4 psum banks of 8 chunks
                            ps = psz.tile([128, 8 * H], f32, tag="z1ps")
                            for j in range(8):
                                ck = 8 * q + j
                                nc.tensor.matmul(
                                    ps[:, H * j:H * (j + 1)],
                                    lhsT=ft_sb[g][:, 128 * ck:128 * (ck + 1)],
                                    rhs=w1_sb[:], start=True, stop=True,
                                )
                            nc.scalar.copy(zt[:, 8 * H * q:8 * H * (q + 1)], ps[:])
                        z1n[g] = zt

                    # ---- layer 1 propagation (local columns) + bias + leaky ----
                    h1_bf = {}
                    for g in "st":
                        psH = psp.tile([H, NP], f32, tag="psH")
                        for c in range(4):
                            warm_fill(6)   # cover the PT-chunk DMA wait
                            for k in range(8 * c, 8 * c + 8):
                                nc.tensor.matmul(
                                    psH[:],
                                    lhsT=z1n[g][:, H * k:H * (k + 1)],
                                    rhs=pt_sb[g][:, NP * k:NP * (k + 1)],
                                    start=(k == 0), stop=(k == 31),
                                )
                        tsb = wp.tile([H, NP], f32, tag="hb")
                        nc.vector.tensor_scalar(tsb[:], psH[:], b1_sb[:], None, Alu.add)
                        hb = gp.tile([H, NP], bf16, tag=f"h1_{g}", name=f"h1_{g}")
                        nc.vector.scalar_tensor_tensor(hb[:], tsb[:], NEG, tsb[:],
                                                       Alu.mult, Alu.max)
                        h1_bf[g] = hb

                    # ---- transpose h1 to node-major, pack, AllGather A ----
                    h1n = gp.tile([128, 2 * 4 * H], bf16, tag="h1n")
                    for gi, g in ((0, "s"), (1, "t")):
                        for b in range(4):
                            psT = psz.tile([128, H], bf16, tag="z1ps", name=f"psT{gi}{b}")
                            nc.tensor.transpose(psT[:], h1_bf[g][:, 128 * b:128 * (b + 1)],
                                                eye_sb[:])
                            nc.scalar.copy(h1n[:, (gi * 4 + b) * H:(gi * 4 + b + 1) * H],
                                           psT[:])
                    nc.sync.dma_start(out=agA_in.ap(), in_=h1n[:])
                    nc.gpsimd.collective_compute(
                        "AllGather", Alu.bypass, replica_groups=RG,
                        ins=[agA_in.ap()], outs=[agA_out.ap()],
                    )

                    # PE warm chain B through the collective wait
                    for w in range(40):
                        nc.tensor.matmul(wps[:], lhsT=warm_src[:, 0:H],
                                         rhs=warm_src[:], start=False,
                                         stop=False, skip_group_check=True)

                    # ---- layer 2: gather z, propagate, apply W2, bias, leaky ----
                    engs = [nc.sync, nc.scalar, nc.gpsimd]
                    z_tiles = []
                    for r in range(8):
                        zr = gp.tile([128, 2 * 4 * H], bf16, tag=f"zr{r}",
                                     name=f"zr{r}")
                        engs[r % 3].dma_start(out=zr[:], in_=agA_out.ap()[r])
                        z_tiles.append(zr)
                    for gi, g in ((0, "s"), (1, "t")):
                        psA = psp.tile([H, NP], f32, tag="psH", name=f"psA_{g}")
                        for k in range(32):
                            zsrc = z_tiles[k // 4]
                            off = gi * 4 * H + (k % 4) * H
                            nc.tensor.matmul(
                                psA[:],
                                lhsT=zsrc[:, off:off + H],
                                rhs=pt_sb[g][:, NP * k:NP * (k + 1)],
                                start=(k == 0), stop=(k == 31),
                            )
                        aA = wp.tile([H, NP], bf16, tag="aA")
                        nc.vector.tensor_copy(aA[:], psA[:])
                        ps2 = psp.tile([H, NP], f32, tag="psH", name=f"ps2_{g}")
                        nc.tensor.matmul(ps2[:], lhsT=w2_sb[:], rhs=aA[:],
                                         start=True, stop=True)
                        tsb = wp.tile([H, NP], f32, tag="hb", name=f"hb2_{g}")
                        nc.vector.tensor_scalar(tsb[:], ps2[:], b2_sb[:], None, Alu.add)
                        nc.vector.scalar_tensor_tensor(h2_bf[g][:], tsb[:], NEG, tsb[:],
                                                       Alu.mult, Alu.max)

            # ============ stats + AllGather B ============
            with nc.named_scope("stats_agB"):
                with tc.tile_pool(name="ps_stat", bufs=2, space="PSUM") as psst, \
                     tc.tile_pool(name="ps_warm2", bufs=1, space="PSUM") as psw2:
                    sq_bf = pp.tile([1, 2 * NP], bf16, tag="sq_bf")
                    s1p = pp.tile([1, 2], f32, tag="s1p")
                    vpg = pp.tile([H, 2], f32, tag="vpg")
                    for gi, g in ((0, "s"), (1, "t")):
                        hsq = wp.tile([H, NP], bf16, tag="hsq")
                        nc.vector.tensor_tensor(hsq[:], h2_bf[g][:], h2_bf[g][:], Alu.mult)
                        psq = psst.tile([1, NP], f32, tag="stat")
                        nc.tensor.matmul(psq[:], lhsT=ones64[:], rhs=hsq[:],
                                         start=True, stop=True)
                        nc.scalar.activation(sq_bf[:, gi * NP:(gi + 1) * NP],
                                             psq[:], Act.Copy,
                                             accum_out=s1p[:, gi:gi + 1])
                        vscr = wp.tile([H, NP], f32, tag="vscr")
                        nc.vector.tensor_scalar(vscr[:], h2_bf[g][:], 0.0, 0.0, Alu.add,
                                                Alu.add, accum_out=vpg[:, gi:gi + 1])
                    v_part = pp.tile([H, 1], f32, tag="v_part")
                    nc.vector.tensor_reduce(v_part[:], vpg[:], AxX, Alu.add)

                    # pack payload: h2 s|t, sq, f32 stats
                    for gi, g in ((0, "s"), (1, "t")):
                        nc.sync.dma_start(
                            out=agB_in.ap()[:, gi * H * NP:(gi + 1) * H * NP]
                                .rearrange("o (f j) -> (o f) j", f=H),
                            in_=h2_bf[g][:])
                    nc.sync.dma_start(out=agB_in.ap()[:, SQ_OFF:SQ_OFF + 2 * NP],
                                      in_=sq_bf[:])
                    nc.sync.dma_start(
                        out=agB_in.ap()[:, F32_OFF:F32_OFF + 4].bitcast(f32),
                        in_=s1p[:])
                    nc.sync.dma_start(
                        out=agB_in.ap()[:, F32_OFF + 4:F32_OFF + 4 + 2 * H].bitcast(f32),
                        in_=v_part[:])
                    nc.gpsimd.collective_compute(
                        "AllGather", Alu.bypass, replica_groups=RG,
                        ins=[agB_in.ap()], outs=[agB_out.ap()],
                    )

                    # PE warm chain C + classifier during the collective
                    wps2 = psw2.tile([H, NP], f32, tag="warm2")
                    for w in range(48):
                        nc.tensor.matmul(wps2[:], lhsT=warm_src[:, 0:H],
                                         rhs=warm_src[:], start=(w == 0),
                                         stop=False, skip_group_check=True)

                    nc.vector.tensor_copy(cls_lhsT[0:H, :], h2_bf["s"][:])
                    for b in range(4):
                        psL = psst.tile([128, C], f32, tag="cls")
                        nc.tensor.matmul(psL[:], lhsT=cls_lhsT[:, 128 * b:128 * (b + 1)],
                                         rhs=fca_sb[:], start=True, stop=True)
                        esc = wp.tile([128, C], f32, tag="cls_t")
                        nc.scalar.activation(esc[:], psL[:], Act.Exp,
                                             accum_out=rgrid[:, 34 + b:35 + b])
                        pks = wp.tile([128, C], f32, tag="cls_t")
                        nc.vector.scalar_tensor_tensor(
                            pks[:], psL[:], 0.0, oh_sb[:, C * b:C * (b + 1)],
                            Alu.add, Alu.mult, accum_out=rgrid[:, 38 + b:39 + b],
                        )

            # =================== MMD phase ===================
            mp_cm = tc.tile_pool(name="mmd", bufs=1)
            mp = mp_cm.__enter__()
            with nc.named_scope("mmd_prep"):
                    st_f32 = agB_out.ap().bitcast(f32)  # [NCORES, 1, AGW_B//2]
                    FB = F32_OFF // 2

                    # ---- rhs: stage raw gathered rows in SBUF, write doubled ----
                    rhs_aug = mp.tile([K_AUG, M2], bf16, tag="rhs_aug")
                    for g in range(2):
                        nc.sync.dma_start(
                            out=rhs_aug[0:H, g * N:(g + 1) * N]
                                .rearrange("f (r j) -> f r j", r=NCORES),
                            in_=agB_out.ap()[:, :, g * H * NP:(g + 1) * H * NP]
                                .rearrange("r o (f j) -> (o f) r j", f=H),
                        )
                    nc.scalar.dma_start(
                        out=rhs_aug[H:H + 1, :], in_=ones16k_d.ap()[:, 0:M2])
                    nc.scalar.dma_start(
                        out=rhs_aug[H + 1:H + 2, :]
                            .rearrange("o (g r j) -> o g r j", g=2, r=NCORES),
                        in_=agB_out.ap()[:, :, SQ_OFF:SQ_OFF + 2 * NP]
                            .rearrange("r o (g j) -> o g r j", g=2),
                    )
                    nc.sync.dma_start(out=rhs_dram.ap()[:, 0:M2], in_=rhs_aug[:])
                    nc.scalar.dma_start(out=rhs_dram.ap()[:, M2:2 * M2], in_=rhs_aug[:])

                    # ---- global stats -> c ----
                    s1g = mp.tile([1, NCORES * 2], f32, tag="s1g")
                    nc.sync.dma_start(
                        out=s1g[:].rearrange("o (r c) -> o r c", r=NCORES),
                        in_=st_f32[:, :, FB:FB + 2].rearrange("r o c -> o r c"),
                    )
                    s1_all = mp.tile([1, 1], f32, tag="s1_all")
                    nc.vector.tensor_reduce(s1_all[:], s1g[:], AxX, Alu.add)
                    vg = mp.tile([H, NCORES], f32, tag="vg")
                    nc.sync.dma_start(
                        out=vg[:],
                        in_=st_f32[:, :, FB + 2:FB + 2 + H].rearrange("r o f -> (o f) r"),
                    )
                    v_sb = mp.tile([H, 1], f32, tag="v_sb")
                    nc.vector.tensor_reduce(v_sb[:], vg[:], AxX, Alu.add)
                    v2_sb = mp.tile([H, 1], f32, tag="v2_sb")
                    nc.vector.tensor_tensor(v2_sb[:], v_sb[:], v_sb[:], Alu.mult)
                    vv_all = mp.tile([H, 1], f32, tag="vv_all")
                    nc.gpsimd.partition_all_reduce(vv_all[:], v2_sb[:], channels=H,
                                                   reduce_op=bass_isa.ReduceOp.add)
                    # bwsum = 2*m*S1 - 2*vv ; bw_base = bwsum/(m^2-m)/4 ; c = 1/(16*bw_base)
                    sc_s1 = mp.tile([1, 1], f32, tag="sc_s1")
                    nc.vector.tensor_scalar(sc_s1[:], s1_all[:], float(2 * M2), None,
                                            Alu.mult)
                    sc_bw = mp.tile([1, 1], f32, tag="sc_bw")
                    nc.vector.scalar_tensor_tensor(sc_bw[:], vv_all[0:1, :], -2.0,
                                                   sc_s1[:], Alu.mult, Alu.add)
                    denom = float(M2) * float(M2 - 1) * 4.0
                    nc.vector.tensor_scalar(sc_bw[:], sc_bw[:], 1.0 / denom, None,
                                            Alu.mult)
                    sc_inv = mp.tile([1, 1], f32, tag="sc_inv")
                    nc.vector.reciprocal(sc_inv[:], sc_bw[:])
                    nc.vector.tensor_scalar(sc_inv[:], sc_inv[:], 1.0 / 16.0, None,
                                            Alu.mult)
                    cb = mp.tile([128, 1], f32, tag="cb")
                    nc.gpsimd.partition_broadcast(cb[:], sc_inv[:])
                    c2col = mp.tile([128, 1], f32, tag="c2col")
                    nc.vector.tensor_scalar(c2col[:], cb[:], 2.0, None, Alu.mult)
                    ncol = mp.tile([128, 1], f32, tag="ncol")
                    nc.vector.tensor_scalar(ncol[:], cb[:], -1.0, None, Alu.mult)

                    # ---- rotated rhs read (dynamic offset, 4 chunks) ----
                    rhs_rot = mp.tile([K_AUG, M2], bf16, tag="rhs_rot")
                    for ch in range(4):
                        eng, off = ((nc.sync, rot_off) if ch % 2 == 0
                                    else (nc.scalar, rot_off2))
                        eng.dma_start(
                            out=rhs_rot[:, 2048 * ch:2048 * (ch + 1)],
                            in_=rhs_dram.ap()[:, bass.ds(off + 2048 * ch, 2048)],
                        )

                    # ---- lhs: c-scaled local rows (aug rows via partition-0 + DMA) ----
                    lhsT_aug = mp.tile([K_AUG, 2 * NP], bf16, tag="lhsT_aug")
                    for gi, g in ((0, "s"), (1, "t")):
                        nc.vector.tensor_scalar(lhsT_aug[0:H, gi * NP:(gi + 1) * NP],
                                                h2_bf[g][:], c2col[0:H, :], None,
                                                Alu.mult)
                    lsqn = mp.tile([1, 2 * NP], bf16, tag="lsqn")
                    nc.vector.tensor_scalar(lsqn[:], sq_bf[:], ncol[0:1, :], None,
                                            Alu.mult)
                    nc.sync.dma_start(out=lhsT_aug[H:H + 1, :], in_=lsqn[:])
                    lones = mp.tile([1, 2 * NP], bf16, tag="lones")
                    nc.vector.tensor_scalar(lones[:], ones1k_sb[:], ncol[0:1, :], None,
                                            Alu.mult)
                    nc.scalar.dma_start(out=lhsT_aug[H + 1:H + 2, :], in_=lones[:])

            with nc.named_scope("mmd_loop"):
                with tc.tile_pool(name="u_scr", bufs=3) as scr, \
                     tc.tile_pool(name="u2p", bufs=3) as u2p, \
                     tc.tile_pool(name="u4p", bufs=3) as u4p, \
                     tc.tile_pool(name="u8p", bufs=3) as u8p, \
                     tc.tile_pool(name="u16p", bufs=3) as u16p, \
                     tc.tile_pool(name="ps_q", bufs=3, space="PSUM") as psq, \
                     tc.tile_pool(name="ps_acc", bufs=1, space="PSUM") as psa:

                    # persistent pm-weighted accumulator (u2-dve/u4/u8/u16 sums)
                    acc_ps = psa.tile([1, NP], f32, tag="acc")
                    first_acc = [True]

                    def acc_reduce(utile, qi):
                        for t in range(2):
                            nc.tensor.matmul(
                                acc_ps[:], lhsT=pm_sb[:, 2 * qi + t:2 * qi + t + 1],
                                rhs=utile[:, NP * t:NP * (t + 1)],
                                start=first_acc[0], stop=False,
                                skip_group_check=True,
                            )
                            first_acc[0] = False

                    qi = 0
                    for half in range(2):
                        xs = range(0, 9) if half == 0 else range(8, 16)
                        its = (0, 2) if half == 0 else (4, 6)
                        for x in xs:
                            psG = psq.tile([128, 2 * NP], f32, tag="psG")
                            for t, it in enumerate(its):
                                nc.tensor.matmul(
                                    psG[:, NP * t:NP * (t + 1)],
                                    lhsT=lhsT_aug[:, 128 * it:128 * (it + 1)],
                                    rhs=rhs_rot[:, NP * x:NP * (x + 1)],
                                    start=True, stop=True,
                                )
                            u1 = scr.tile([128, 2 * NP], bf16, tag="u1")
                            nc.scalar.activation(
                                u1[:], psG[:], Act.Exp,
                                accum_out=rgrid[:, 2 * qi:2 * qi + 1])
                            u2 = u2p.tile([128, 2 * NP], bf16, tag="u2")
                            if qi in DVE_U2:
                                nc.vector.tensor_tensor(u2[:], u1[:], u1[:], Alu.mult)
                                acc_reduce(u2, qi)
                            else:
                                nc.scalar.activation(
                                    u2[:], psG[:], Act.Exp, scale=2.0,
                                    accum_out=rgrid[:, 2 * qi + 1:2 * qi + 2])
                            u4 = u4p.tile([128, 2 * NP], bf16, tag="u4")
                            nc.vector.tensor_tensor(u4[:], u2[:], u2[:], Alu.mult)
                            acc_reduce(u4, qi)
                            u8 = u8p.tile([128, 2 * NP], bf16, tag="u8")
                            nc.vector.tensor_tensor(u8[:], u4[:], u4[:], Alu.mult)
                            acc_reduce(u8, qi)
                            u16 = u16p.tile([128, 2 * NP], bf16, tag="u16")
                            nc.vector.tensor_tensor(u16[:], u8[:], u8[:], Alu.mult)
                            acc_reduce(u16, qi)
                            qi += 1

                    acc_sb = scr.tile([1, NP], f32, tag="acc_sb")
                    nc.scalar.activation(acc_sb[:], acc_ps[:], Act.Copy,
                                         accum_out=rgrid[0:1, 93:94])

            mp_cm.__exit__(None, None, None)
            nc.sync.dma_start(out=out_d.ap(), in_=rgrid[:])

    nc.compile()
    return nc


def _host_prep(inputs):
    """Build PT matrices + per-core input shards."""
    fs = np.ascontiguousarray(np.asarray(inputs["features_s"], np.float32))
    ft = np.ascontiguousarray(np.asarray(inputs["features_t"], np.float32))
    W1 = np.asarray(inputs["W1"], np.float32)
    W2 = np.asarray(inputs["W2"], np.float32)
    b1 = np.asarray(inputs["b1"], np.float32).reshape(H, 1)
    b2 = np.asarray(inputs["b2"], np.float32).reshape(H, 1)
    fc_w = np.asarray(inputs["fc_w"], np.float32)
    fc_b = np.asarray(inputs["fc_b"], np.float32)
    labels = np.asarray(inputs["labels_s"]).astype(np.int64)

    def build_PT(src, dst):
        src = np.asarray(src).astype(np.int64)
        dst = np.asarray(dst).astype(np.int64)
        deg = np.bincount(dst, minlength=N).astype(np.float32) + 1.0
        norm = 1.0 / np.sqrt(deg)
        AT = np.bincount(src * N + dst, minlength=N * N).astype(np.float32).reshape(N, N)
        AT[np.arange(N), np.arange(N)] += 1.0
        PT = AT * norm[None, :]
        PT *= norm[:, None]
        return PT

    PTs = build_PT(inputs["es_src"], inputs["es_dst"])
    PTt = build_PT(inputs["et_src"], inputs["et_dst"])

    fc_aug = np.concatenate([fc_w, fc_b[None, :]], axis=0).astype(BF16)
    eye = np.eye(H, dtype=np.float32).astype(BF16)

    onehot = np.zeros((N, C), np.float32)
    onehot[np.arange(N), labels] = 1.0

    ftS_T = np.ascontiguousarray(fs.T).astype(BF16)
    ftT_T = np.ascontiguousarray(ft.T).astype(BF16)
    ones16k = np.ones((1, 2 * M2), BF16)
    ones1k = np.ones((1, 2 * NP), BF16)

    in_maps = []
    for r in range(NCORES):
        sl = slice(NP * r, NP * (r + 1))
        oh_r = onehot[sl].reshape(4, 128, C).transpose(1, 0, 2).reshape(128, 4 * C)
        in_maps.append({
            "colbase": np.array([[NP * r]], np.int32),
            "ftS": ftS_T, "ftT": ftT_T,
            "ptS": np.ascontiguousarray(PTs[:, sl]).astype(BF16),
            "ptT": np.ascontiguousarray(PTt[:, sl]).astype(BF16),
            "w1b": W1.astype(BF16), "w2b": W2.astype(BF16),
            "b1": b1, "b2": b2,
            "fca": fc_aug,
            "oh": np.ascontiguousarray(oh_r),
            "eye": eye,
            "ones16k": ones16k, "ones1k": ones1k,
            "pm_all": np.ascontiguousarray(
                np.broadcast_to(np.repeat(2.0 * _quad_weights(r), 2), (128, 34))
            ).astype(BF16),
        })
    return in_maps


def _quad_weights(r):
    """Symmetry weight for each of the 17 quads on core r (host side)."""
    w = np.zeros(NQUAD, np.float64)
    qi = 0
    for half in range(2):
        xs = range(0, 9) if half == 0 else range(8, 16)
        A = r if half == 0 else r + 8
        si = 1.0 if half == 0 else -1.0
        for x in xs:
            G = (r + x) % 16
            sj = 1.0 if G < 8 else -1.0
            diag = ((G - A) % 16 == 0)
            w[qi] = si * sj * (1.0 if diag else 2.0)
            qi += 1
    return w


def kernel(**inputs):
    global LAST_EXEC_NS, LAST_SCOPES
    from concourse.bass_utils import run_bass_kernel_spmd

    trace = bool(int(os.environ.get("KBENCH_TRACE", "0")))
    if trace:
        _install_ntff_hook()

    if "nc" not in _CACHE:
        _CACHE["nc"] = _build_program()
    nc = _CACHE["nc"]

    in_maps = _host_prep(inputs)
    res = run_bass_kernel_spmd(nc, in_maps, list(range(NCORES)), trace=trace)
    LAST_EXEC_NS = res.exec_time_ns
    LAST_SCOPES = res.per_core_scope_times

    mmd_total = 0.0
    pk_total = 0.0
    lse_total = 0.0
    for r in range(NCORES):
        out = res.results[r]["out_vec"].astype(np.float64)
        w = 2.0 * _quad_weights(r)
        for q in range(NQUAD):
            mmd_total += w[q] * out[:, 2 * q:2 * q + 2].sum()
        mmd_total += out[:, 93].sum()
        se = out[:, 34:38]
        pk = out[:, 38:42]
        lse_total += np.log(se).sum()
        pk_total += pk.sum()
    class_loss = -(pk_total - lse_total) / N
    domain_loss = mmd_total / (N * N)
    return np.float32(class_loss + 0.5 * domain_loss)
